# revision 1
# baseline (speedup 1.0000x reference)
"""Trainium2 Bass kernel for nn_DecoderBlock (Autoformer decoder layer).

Data-parallel over batch: 16 batches -> 8 cores x 2 batches, per-batch
serial pipeline on each core. Layout is channel-major (E on partitions).

Autocorrelation without FFTs:
  M[tau] = (1/E) sum_t <k_t, q_{(t+tau)%L}>
via 128-row tiles of K^T Q whose wrapped diagonals are summed by a DRAM
shear round-trip (row stride 1153 on readback) + ones-matmul reduction.
Top-6 + softmax give a scattered weight vector; the roll-aggregation
  agg[e,t] = sum_{s'} Vrev[s',e] * c2R[s'+t]
uses host-time-reversed V inputs and the reversed weight vector written
twice to DRAM, so all access patterns are positive-stride.

Precision: score matmuls in true fp32 (top-6 margins ~1e-4 in M units);
FFN in bf16 (its output is ~0.2x the residual scale, so bf16 noise
dilutes well below tolerance); everything else float32r.
"""
import functools
import numpy as np

NCORES = 8
BPC = 2
L = 1024
E = 512
XP = 2048
F = 512
KS = 25
NCH = 4
NT = 8
NXP = 16
SCORES_F32R = True


@functools.lru_cache(maxsize=1)
def _build():
    import concourse.bacc as bacc
    import concourse.bass as bass
    import concourse.mybir as mybir
    from concourse.tile import TileContext

    F32 = mybir.dt.float32
    F32R = mybir.dt.float32r
    BF16 = mybir.dt.bfloat16
    AF = mybir.ActivationFunctionType
    OP = mybir.AluOpType

    SCORE_DT = F32R if SCORES_F32R else F32
    nc = bacc.Bacc("TRN2", target_bir_lowering=False, debug=False, num_devices=NCORES)

    def din(name, shape, dtype=F32R):
        return nc.declare_dram_parameter(name, list(shape), dtype, isOutput=False)

    xt_in = din("xt", (BPC, E, L))
    xtr_in = din("xtr", (BPC, E, L))
    ent_in = din("ent", (BPC, E, L))
    entr_in = din("entr", (BPC, E, L))
    w_in = {}
    for p in ("sa", "ca"):
        for nme in ("wq", "wk", "wv", "wo"):
            w_in[f"{p}_{nme}"] = din(f"{p}_{nme}", (E, E))
    fcw_in = din("fcw", (128, 16384), BF16)   # packed fc1(c,2048)|fc2(xc,512), bf16
    wct_in = din("wct", (3 * E, F))
    bias_pc_in = din("bias_pc", (128, 52), F32)
    brow_in = din("brow", (1, 2 * E), F32R)
    ident_in = din("ident", (128, 128), F32)
    identr_in = din("identr", (128, 128), F32R)
    jmat_in = din("jmat", (128, 128), F32)
    ones_div_in = din("ones_div", (128, 1))
    ones_row_in = din("ones_row", (1, 128))
    rampl_in = din("rampl", (128, 16), F32)
    ones12_in = din("ones12", (128, 12), F32)

    seas_out = nc.declare_dram_parameter("seasonal", [BPC, L, E], F32, isOutput=True)

    trend_out = nc.declare_dram_parameter("trend", [BPC, L, F], F32, isOutput=True)

    a2d = {(b, p): nc.dram_tensor(f"a2d_{p}{b}", [L, 1152], F32R)
           for b in range(BPC) for p in ("sa", "ca")}
    c2rd = {(b, p): nc.dram_tensor(f"c2rd_{p}{b}", [2 * L], BF16)
            for b in range(BPC) for p in ("sa", "ca")}
    trd = {b: nc.dram_tensor(f"trdram{b}", [NCH, 128, L], F32) for b in range(BPC)}

    BQ = {"sa": 0, "ca": 12}
    BK = {"sa": 4, "ca": 16}
    BO = {"sa": 8, "ca": 20}
    FC2B, LNG, LNB, FC1B = 24, 28, 32, 36
    BVROW = {"sa": 0, "ca": E}

    with TileContext(nc) as tc:
        with (
            tc.tile_pool(name="consts", bufs=1) as cpool,
            tc.tile_pool(name="wst", bufs=4) as wstp,
            tc.tile_pool(name="acts", bufs=1) as apool,
            tc.tile_pool(name="str", bufs=1) as spool,
            tc.tile_pool(name="pp", bufs=2, space="PSUM") as pspool,
            tc.tile_pool(name="pa", bufs=1, space="PSUM") as papool,
            tc.tile_pool(name="pm", bufs=1, space="PSUM") as pmpool,
        ):
            ident = cpool.tile([128, 128], F32, name="ident")
            identr = cpool.tile([128, 128], F32R, name="identr")
            jmat = cpool.tile([128, 128], F32, name="jmat")
            ones_div = cpool.tile([128, 1], F32R, name="ones_div")
            ones_row = cpool.tile([1, 128], F32R, name="ones_row")
            rampl = cpool.tile([128, 16], F32, name="rampl")
            ones12 = cpool.tile([128, 12], F32, name="ones12")
            bias_pc = cpool.tile([128, 52], F32, name="bias_pc")
            brow = cpool.tile([1, 2 * E], F32R, name="brow")
            for t, d in ((ident, ident_in), (identr, identr_in), (jmat, jmat_in),
                         (ones_div, ones_div_in), (ones_row, ones_row_in),
                         (rampl, rampl_in), (ones12, ones12_in),
                         (bias_pc, bias_pc_in), (brow, brow_in)):
                nc.sync.dma_start(out=t[:], in_=d.ap())

            def ps_tile(name):
                return pspool.tile([128, 512], F32, name=name, tag="pp")

            def wchunk(dram, sl0, sl1, name):
                """stream a (128, 512) f32r weight chunk: rows sl0 (c-range), cols sl1."""
                t = wstp.tile([128, 512], F32R, name=name, tag="wst")
                nc.sync.dma_start(
                    out=t[:].rearrange("p (c n) -> p c n", c=(sl0.stop - sl0.start))
                    if False else t[:],
                    in_=dram.ap().rearrange("(c p) n -> p c n", p=128)[:, sl0, sl1]
                    .rearrange("p c n -> p (c n)"))
                return t

            def proj_el(out_t, src_t, w_dram, bcol, resid=None, resid_dram=None):
                """out[e_out,t] = sum_e w[e,e_out] src[e,t] + bias (+resid)."""
                for m in range(NCH):
                    wm = wstp.tile([128, 512], F32R, name=f"wm{m}", tag="wst")
                    nc.sync.dma_start(
                        out=wm[:].rearrange("p (c n) -> p c n", c=NCH),
                        in_=w_dram.ap().rearrange("(c p) n -> p c n", p=128)
                            [:, :, 128 * m : 128 * m + 128])
                    for h in range(2):
                        pt = ps_tile(f"pp{m}{h}")
                        first = True
                        if resid is not None:
                            nc.tensor.matmul(pt[:], identr[:],
                                             resid[:, m * L + 512 * h : m * L + 512 * h + 512],
                                             start=True, stop=False)
                            first = False
                        elif resid_dram is not None:
                            rs = wstp.tile([128, 512], F32R, name=f"rs{m}{h}", tag="wst")
                            nc.sync.dma_start(
                                out=rs[:],
                                in_=resid_dram.rearrange("(c p) l -> p c l", p=128)
                                    [:, m, 512 * h : 512 * h + 512])
                            nc.tensor.matmul(pt[:], identr[:], rs[:], start=True, stop=False)
                            first = False
                        for c in range(NCH):
                            nc.tensor.matmul(
                                pt[:], wm[:, c * 128 : c * 128 + 128],
                                src_t[:, c * L + 512 * h : c * L + 512 * h + 512],
                                start=(first and c == 0), stop=(c == NCH - 1))
                        dst = out_t[:, m * L + 512 * h : m * L + 512 * h + 512]
                        if (m + h) % 2 == 0:
                            nc.vector.tensor_scalar(dst, pt[:],
                                bias_pc[:, bcol + m : bcol + m + 1], None, op0=OP.add)
                        else:
                            nc.scalar.activation(dst, pt[:], AF.Identity,
                                bias=bias_pc[:, bcol + m : bcol + m + 1])

            def proj_rev(out_t, src_rev_t, w_dram, bvcol):
                """time-reversed V in (L,E): out[s',e] = sum_e' xrev[e',s'] w[e',e] + bv."""
                wv = []
                for c in range(NCH):
                    wc = wstp.tile([128, 512], F32R, name=f"wvc{c}", tag="wst")
                    nc.sync.dma_start(
                        out=wc[:],
                        in_=w_dram.ap().rearrange("(c p) n -> p c n", p=128)[:, c, :])
                    wv.append(wc)
                for j in range(NT):
                    pt = ps_tile(f"pv{j}")
                    nc.tensor.matmul(pt[:], ones_row[:], brow[0:1, bvcol : bvcol + E],
                                     start=True, stop=False)
                    for c in range(NCH):
                        nc.tensor.matmul(
                            pt[:], src_rev_t[:, c * L + 128 * j : c * L + 128 * j + 128],
                            wv[c][:], start=False, stop=(c == NCH - 1))
                    if j % 2 == 0:
                        nc.vector.tensor_copy(out_t[:, j * E : (j + 1) * E], pt[:])
                    else:
                        nc.scalar.activation(out_t[:, j * E : (j + 1) * E], pt[:], AF.Copy)

            def decomp(b, y_t, xnext_t, first_tr):
                """xnext = y - movavg25(y); DRAM trend (+)= movavg25(y)."""
                for c in range(NCH):
                    y = lambda a, bb: y_t[:, c * L + a : c * L + bb]
                    ws = spool.tile([128, L], F32, name=f"ws{c}", tag="ws", bufs=2)
                    cs25 = spool.tile([128, 25], F32, name=f"cs25{c}", tag="cs25", bufs=1)
                    nc.vector.tensor_tensor_scan(cs25[:], y(0, 25), y(0, 25), 0.0,
                                                 op0=OP.add, op1=OP.bypass)
                    nc.vector.scalar_tensor_tensor(ws[:, 0:13], rampl[:, 0:13], y(0, 1),
                                                   cs25[:, 12:25], op0=OP.mult, op1=OP.add)
                    nc.vector.tensor_tensor_scan(ws[:, 13:1012], y(25, L), y(0, 999),
                                                 cs25[:, 24:25], op0=OP.add, op1=OP.subtract)
                    ylast = spool.tile([128, 12], F32, name=f"yl{c}", tag="yl", bufs=2)
                    nc.vector.tensor_scalar(ylast[:], ones12[:], y(L - 1, L), None, op0=OP.mult)
                    nc.vector.tensor_tensor_scan(ws[:, 1012:1024], ylast[:], y(999, 1011),
                                                 ws[:, 1011:1012], op0=OP.add, op1=OP.subtract)
                    nc.vector.scalar_tensor_tensor(
                        xnext_t[:, c * L : (c + 1) * L], ws[:], -1.0 / KS, y(0, L),
                        op0=OP.mult, op1=OP.add)
                    # trend accumulation in DRAM: ws *= 1/25 then (accum-)DMA out
                    nc.vector.tensor_scalar(ws[:], ws[:], 1.0 / KS, None, op0=OP.mult)
                    if first_tr:
                        nc.gpsimd.dma_start(out=trd[b].ap()[c], in_=ws[:])
                    else:
                        nc.gpsimd.dma_start(out=trd[b].ap()[c], in_=ws[:],
                                            accum_op=OP.add)

            def attention(b, pfx, xq_t, kv_t, kvrev_t, out_y_t, resid=None, resid_dram=None):
                q_t = apool.tile([128, NCH * L], SCORE_DT, name=f"q{pfx}{b}", tag="q", bufs=1)
                k_t = apool.tile([128, NCH * L], SCORE_DT, name=f"k{pfx}{b}", tag="big32", bufs=1)
                proj_el(q_t, xq_t, w_in[f"{pfx}_wq"], BQ[pfx])
                proj_el(k_t, kv_t, w_in[f"{pfx}_wk"], BK[pfx])

                # ---- scores (fp32)
                a2 = a2d[(b, pfx)]
                m_ps = pmpool.tile([1, L], F32, name=f"mps{pfx}{b}", tag="psbig")
                for i in range(NT):
                    # pa banks: h=0 at 0, h=1 at 512, h=2 at 1024; the (at most
                    # one) wrapped split piece goes to its own bank at 1536 so
                    # every accumulation group has exactly one start=True owner.
                    pa = papool.tile([128, 2048], F32, name=f"pa{i}", tag="pa")
                    split_h, split_n1 = -1, 0
                    for c in range(NCH):
                        lhs = k_t[:, c * L + 128 * i : c * L + 128 * i + 128]
                        for h in range(3):
                            w0 = 128 * i + 384 * h
                            st, sp = (c == 0), (c == NCH - 1)
                            o = 512 * h
                            if w0 + 384 <= L:
                                nc.tensor.matmul(pa[:, o : o + 384], lhs,
                                                 q_t[:, c * L + w0 : c * L + w0 + 384],
                                                 start=st, stop=sp)
                            elif w0 >= L:
                                nc.tensor.matmul(pa[:, o : o + 384], lhs,
                                                 q_t[:, c * L + w0 - L : c * L + w0 - L + 384],
                                                 start=st, stop=sp)
                            else:
                                n1 = L - w0
                                split_h, split_n1 = h, n1
                                nc.tensor.matmul(pa[:, o : o + n1], lhs,
                                                 q_t[:, c * L + w0 : c * L + L],
                                                 start=st, stop=sp)
                                nc.tensor.matmul(pa[:, 1536 : 1536 + 384 - n1], lhs,
                                                 q_t[:, c * L : c * L + 384 - n1],
                                                 start=st, stop=sp)
                    a2sb = spool.tile([128, 1152], F32R, name=f"a2sb{i}", tag="a2sb", bufs=1)
                    for h in range(3):
                        if h == split_h:
                            n1 = split_n1
                            nc.vector.tensor_copy(a2sb[:, 384 * h : 384 * h + n1],
                                                  pa[:, 512 * h : 512 * h + n1])
                            nc.vector.tensor_copy(a2sb[:, 384 * h + n1 : 384 * h + 384],
                                                  pa[:, 1536 : 1536 + 384 - n1])
                        else:
                            nc.vector.tensor_copy(a2sb[:, 384 * h : 384 * h + 384],
                                                  pa[:, 512 * h : 512 * h + 384])
                    nc.sync.dma_start(out=a2.ap()[128 * i : 128 * i + 128, :], in_=a2sb[:])
                    sch = spool.tile([128, L], F32R, name=f"sch{i}", tag="ws", bufs=2)
                    nc.sync.dma_start(
                        out=sch[:], in_=bass.AP(a2, 1152 * 128 * i, [[1153, 128], [1, L]]))

                    for h in range(2):
                        nc.tensor.matmul(m_ps[0:1, 512 * h : 512 * h + 512], ones_div[:],
                                         sch[:, 512 * h : 512 * h + 512],
                                         start=(i == 0), stop=(i == NT - 1))

                # ---- V projection (after scores so "v" slot turns over late)
                vrev_t = apool.tile([128, NT * E], BF16, name=f"v{pfx}{b}", tag="v", bufs=1)
                proj_rev(vrev_t, kvrev_t, w_in[f"{pfx}_wv"], BVROW[pfx])

                # ---- reverse M; scattered softmax -> c2R
                m_row = spool.tile([1, L], F32, name=f"mrow{pfx}{b}", tag="mrow", bufs=1)
                nc.vector.tensor_copy(m_row[:], m_ps[:])
                colt_ps = pspool.tile([128, 8], F32, name=f"colt{pfx}{b}", tag="pp")
                for g in range(8):
                    nc.tensor.transpose(colt_ps[:, g : g + 1],
                                        m_row[0:1, 128 * g : 128 * g + 128], ident[0:1, 0:1])
                colt = spool.tile([128, 8], F32, name=f"coltsb{pfx}{b}", tag="coltsb", bufs=1)
                nc.vector.tensor_copy(colt[:], colt_ps[:])
                revt_ps = pspool.tile([128, 8], F32, name=f"revt{pfx}{b}", tag="pp")
                nc.tensor.matmul(revt_ps[:], jmat[:], colt[:], start=True, stop=True)
                revt = spool.tile([128, 8], F32, name=f"revtsb{pfx}{b}", tag="revtsb", bufs=1)
                nc.vector.tensor_copy(revt[:], revt_ps[:])
                mrev_ps = pmpool.tile([1, L], F32, name=f"mrevps{pfx}{b}", tag="psbig")
                for g in range(8):
                    nc.tensor.transpose(mrev_ps[0:1, 128 * (7 - g) : 128 * (7 - g) + 128],
                                        revt[:, g : g + 1], ident[:, :])
                mrev = spool.tile([1, L], F32, name=f"mrev{pfx}{b}", tag="mrev", bufs=1)
                nc.vector.tensor_copy(mrev[:], mrev_ps[:])

                top8 = spool.tile([1, 8], F32, name=f"top8{pfx}{b}", tag="top8", bufs=1)
                nc.vector.max(top8[:], mrev[:])
                negmax = spool.tile([1, 1], F32, name=f"ngm{pfx}{b}", tag="ngm", bufs=1)
                nc.vector.tensor_scalar(negmax[:], top8[:, 0:1], -1.0, None, op0=OP.mult)
                # mask overwrites m_row (dead); exp result then masked in place
                nc.vector.tensor_scalar(m_row[:], mrev[:], top8[:, 5:6], None, op0=OP.is_ge)
                nc.scalar.activation(mrev[:], mrev[:], AF.Exp, bias=negmax[:, 0:1])
                nc.vector.tensor_tensor(mrev[:], m_row[:], mrev[:], OP.mult)
                csum = spool.tile([1, 1], F32, name=f"csum{pfx}{b}", tag="csum", bufs=1)
                nc.vector.tensor_reduce(csum[:], mrev[:], axis=mybir.AxisListType.X, op=OP.add)
                crecip = spool.tile([1, 1], F32, name=f"crec{pfx}{b}", tag="crec", bufs=1)
                nc.vector.reciprocal(crecip[:], csum[:])
                c2r_sb = spool.tile([1, L], BF16, name=f"c2r{pfx}{b}", tag="c2r", bufs=1)
                nc.vector.tensor_scalar(c2r_sb[:], mrev[:], crecip[:, 0:1], None, op0=OP.mult)
                nc.sync.dma_start(out=c2rd[(b, pfx)].ap()[0:L], in_=c2r_sb[:])
                nc.sync.dma_start(out=c2rd[(b, pfx)].ap()[L:], in_=c2r_sb[:])

                # ---- aggregation
                cf = apool.tile([128, NT * L], BF16, name=f"cf{pfx}{b}", tag="big32", bufs=1)
                nc.sync.dma_start(
                    out=cf[:].rearrange("p (j l) -> p j l", j=NT),
                    in_=bass.AP(c2rd[(b, pfx)], 0, [[1, 128], [128, NT], [1, L]]))
                agg_t = apool.tile([128, NCH * L], F32R, name=f"agg{pfx}{b}", tag="q", bufs=1)
                for m in range(NCH):
                    for h in range(2):
                        pt = ps_tile(f"pag{m}{h}")
                        for j in range(NT):
                            nc.tensor.matmul(
                                pt[:], vrev_t[:, j * E + 128 * m : j * E + 128 * m + 128],
                                cf[:, j * L + 512 * h : j * L + 512 * h + 512],
                                start=(j == 0), stop=(j == NT - 1))
                        dsta = agg_t[:, m * L + 512 * h : m * L + 512 * h + 512]
                        if (m + h) % 2 == 0:
                            nc.vector.tensor_copy(dsta, pt[:])
                        else:
                            nc.scalar.activation(dsta, pt[:], AF.Copy)
                proj_el(out_y_t, agg_t, w_in[f"{pfx}_wo"], BO[pfx],
                        resid=resid, resid_dram=resid_dram)

            # ============== main program (stage-major across batches) ==============
            xel_t, xrev_t, y_t, x_t = {}, {}, {}, {}
            for b in range(BPC):
                xel = apool.tile([128, NCH * L], F32R, name=f"xel{b}", tag="xel", bufs=1)
                nc.sync.dma_start(out=xel[:].rearrange("p (c l) -> p c l", c=NCH),
                                  in_=xt_in.ap()[b].rearrange("(c p) l -> p c l", p=128))
                xrev = apool.tile([128, NCH * L], F32R, name=f"xrev{b}", tag="xrev", bufs=1)
                nc.sync.dma_start(out=xrev[:].rearrange("p (c l) -> p c l", c=NCH),
                                  in_=xtr_in.ap()[b].rearrange("(c p) l -> p c l", p=128))
                xel_t[b], xrev_t[b] = xel, xrev
            for b in range(BPC):
                y1 = apool.tile([128, NCH * L], F32, name=f"y1{b}", tag="y", bufs=2)
                attention(b, "sa", xel_t[b], xel_t[b], xrev_t[b], y1,
                          resid_dram=xt_in.ap()[b])
                y_t[b] = y1
            for b in range(BPC):
                x2 = apool.tile([128, NCH * L], F32R, name=f"x2{b}", tag="x", bufs=2)
                decomp(b, y_t[b], x2, True)
                x_t[b] = x2
            for b in range(BPC):
                ent = apool.tile([128, NCH * L], F32R, name=f"ent{b}", tag="xel", bufs=1)
                nc.sync.dma_start(out=ent[:].rearrange("p (c l) -> p c l", c=NCH),
                                  in_=ent_in.ap()[b].rearrange("(c p) l -> p c l", p=128))
                entr = apool.tile([128, NCH * L], F32R, name=f"entr{b}", tag="xrev", bufs=1)
                nc.sync.dma_start(out=entr[:].rearrange("p (c l) -> p c l", c=NCH),
                                  in_=entr_in.ap()[b].rearrange("(c p) l -> p c l", p=128))
                xel_t[b], xrev_t[b] = ent, entr
            for b in range(BPC):
                y2 = apool.tile([128, NCH * L], F32, name=f"y2{b}", tag="y", bufs=2)
                attention(b, "ca", x_t[b], xel_t[b], xrev_t[b], y2, resid=x_t[b])
                y_t[b] = y2
            for b in range(BPC):
                x3 = apool.tile([128, NCH * L], F32R, name=f"x3{b}", tag="x", bufs=2)
                decomp(b, y_t[b], x3, False)
                x_t[b] = x3
            fcw = apool.tile([128, 16384], BF16, name="fcw", tag="big32", bufs=1)
            nc.sync.dma_start(out=fcw[:], in_=fcw_in.ap())
            for b in range(BPC):
                x3 = x_t[b]
                x3bf = apool.tile([128, NCH * L], BF16, name=f"x3bf{b}", tag="xrev", bufs=1)
                nc.vector.tensor_copy(x3bf[:], x3[:])
                y3 = apool.tile([128, NCH * L], F32, name=f"y3{b}", tag="y", bufs=2)
                for half in range(2):
                    h_t = apool.tile([128, NXP * 512], BF16, name=f"h{b}{half}", tag="v", bufs=1)
                    for xc in range(NXP):
                        pt = ps_tile(f"ph{xc}")
                        for c in range(NCH):
                            nc.tensor.matmul(
                                pt[:], fcw[:, c * 2048 + 128 * xc : c * 2048 + 128 * xc + 128],
                                x3bf[:, c * L + 512 * half : c * L + 512 * half + 512],
                                start=(c == 0), stop=(c == NCH - 1))
                        nc.scalar.activation(h_t[:, xc * 512 : (xc + 1) * 512], pt[:],
                                             AF.Gelu, bias=bias_pc[:, FC1B + xc : FC1B + xc + 1])
                    for m in range(NCH):
                        pt = ps_tile(f"pf{m}")
                        for xc in range(NXP):
                            nc.tensor.matmul(
                                pt[:],
                                fcw[:, 8192 + xc * 512 + 128 * m : 8192 + xc * 512 + 128 * m + 128],
                                h_t[:, xc * 512 : (xc + 1) * 512],
                                start=(xc == 0), stop=(xc == NXP - 1))
                        sl = slice(m * L + 512 * half, m * L + 512 * half + 512)
                        nc.vector.scalar_tensor_tensor(
                            y3[:, sl], pt[:], bias_pc[:, FC2B + m : FC2B + m + 1],
                            x3[:, sl], op0=OP.add, op1=OP.add)
                y_t[b] = y3
            for b in range(BPC):
                x4 = apool.tile([128, NCH * L], F32R, name=f"x4{b}", tag="x", bufs=2)
                decomp(b, y_t[b], x4, False)
                x_t[b] = x4
            for b in range(BPC):
                x4 = x_t[b]
                sq = apool.tile([128, NCH * L], F32R, name=f"sq{b}", tag="q", bufs=1)
                for c in range(NCH):
                    nc.scalar.activation(sq[:, c * L : (c + 1) * L],
                                         x4[:, c * L : (c + 1) * L], AF.Square)
                mu_ps = pmpool.tile([1, L], F32, name=f"mups{b}", tag="psbig")
                for h in range(2):
                    for c in range(NCH):
                        nc.tensor.matmul(mu_ps[0:1, 512 * h : 512 * h + 512], ones_div[:],
                                         x4[:, c * L + 512 * h : c * L + 512 * h + 512],
                                         start=(c == 0), stop=(c == NCH - 1))
                mu_r = spool.tile([1, L], F32, name=f"mur{b}", tag="mrow", bufs=1)
                nc.vector.tensor_copy(mu_r[:], mu_ps[:])
                ms_ps = pmpool.tile([1, L], F32, name=f"msps{b}", tag="psbig")
                for h in range(2):
                    for c in range(NCH):
                        nc.tensor.matmul(ms_ps[0:1, 512 * h : 512 * h + 512], ones_div[:],
                                         sq[:, c * L + 512 * h : c * L + 512 * h + 512],
                                         start=(c == 0), stop=(c == NCH - 1))
                var_r = spool.tile([1, L], F32, name=f"varr{b}", tag="mrev", bufs=1)
                nc.vector.tensor_tensor(var_r[:], mu_r[:], mu_r[:], OP.mult)
                nc.vector.scalar_tensor_tensor(var_r[:], ms_ps[:], 1e-5, var_r[:],
                                               op0=OP.add, op1=OP.subtract)
                nc.scalar.activation(var_r[:], var_r[:], AF.Sqrt)
                rows = spool.tile([1, L], F32R, name=f"rows{b}", tag="c2r", bufs=1)
                rows2 = spool.tile([1, L], F32R, name=f"rows2{b}", tag="rows2", bufs=1)
                with nc.allow_low_precision(reason="istd broadcast is f32r by design"):
                    nc.vector.reciprocal(rows[:], var_r[:])
                nc.vector.tensor_tensor(rows2[:], mu_r[:], rows[:], OP.mult)
                bc = apool.tile([128, 2 * L], F32, name=f"bc{b}", tag="xrev", bufs=1)
                for h in range(4):
                    bp = ps_tile(f"bc{h}")
                    src_row = rows if h < 2 else rows2
                    nc.tensor.matmul(bp[:], ones_row[:],
                                     src_row[0:1, 512 * (h % 2) : 512 * (h % 2) + 512],
                                     start=True, stop=True)
                    nc.vector.tensor_copy(bc[:, 512 * h : 512 * h + 512], bp[:])
                seas = apool.tile([128, NCH * L], F32, name=f"seas{b}", tag="q", bufs=1)
                accs = spool.tile([128, NCH], F32, name=f"accs{b}", tag="accs", bufs=1)
                for c in range(NCH):
                    t1 = spool.tile([128, L], F32, name=f"t1{c}", tag="ws", bufs=2)
                    nc.vector.tensor_tensor(t1[:], x4[:, c * L : (c + 1) * L],
                                            bc[:, 0:L], OP.mult)
                    nc.vector.tensor_tensor(t1[:], t1[:], bc[:, L:], OP.subtract)
                    nc.scalar.activation(seas[:, c * L : (c + 1) * L], t1[:], AF.Identity,
                                         bias=bias_pc[:, LNB + c : LNB + c + 1],
                                         scale=bias_pc[:, LNG + c : LNG + c + 1],
                                         accum_out=accs[:, c : c + 1])
                for c in range(NCH):
                    nc.vector.tensor_scalar(accs[:, c : c + 1], accs[:, c : c + 1],
                                            1.0 / L, None, op0=OP.mult)
                    nc.vector.tensor_scalar(seas[:, c * L : (c + 1) * L],
                                            seas[:, c * L : (c + 1) * L],
                                            accs[:, c : c + 1], None, op0=OP.subtract)
                for a in range(NT):
                    tp = ps_tile(f"tps{a}")
                    for c in range(NCH):
                        nc.tensor.transpose(tp[:, 128 * c : 128 * c + 128],
                                            seas[:, c * L + 128 * a : c * L + 128 * a + 128],
                                            ident[:, :])
                    osb = spool.tile([128, 512], F32, name=f"osb{a}", tag="osb", bufs=2)
                    nc.vector.tensor_copy(osb[:], tp[:])
                    nc.sync.dma_start(out=seas_out.ap()[b, 128 * a : 128 * a + 128, :],
                                      in_=osb[:])
            wct = apool.tile([128, 12 * F], F32R, name="wctt", tag="big32", bufs=1)
            nc.sync.dma_start(out=wct[:].rearrange("p (c n) -> p c n", c=12),
                              in_=wct_in.ap().rearrange("(c p) n -> p c n", p=128))
            for b in range(BPC):
                tpad = apool.tile([128, NCH * 1026], F32R, name=f"tpad{b}", tag="y", bufs=2)
                nc.sync.dma_start(
                    out=tpad[:].rearrange("p (c l) -> p c l", c=NCH)[:, :, 1:1025],
                    in_=trd[b].ap().bitcast(F32R).rearrange("c p l -> p c l"))
                for c in range(NCH):
                    nc.vector.tensor_copy(tpad[:, c * 1026 : c * 1026 + 1],
                                          tpad[:, c * 1026 + L : c * 1026 + L + 1])
                    nc.vector.tensor_copy(tpad[:, c * 1026 + 1025 : c * 1026 + 1026],
                                          tpad[:, c * 1026 + 1 : c * 1026 + 2])
                for a in range(NT):
                    pt = ps_tile(f"ptc{a}")
                    n = 0
                    for j in range(3):
                        for c in range(NCH):
                            nc.tensor.matmul(
                                pt[:],
                                tpad[:, c * 1026 + 128 * a + j : c * 1026 + 128 * a + j + 128],
                                wct[:, (j * NCH + c) * F : (j * NCH + c) * F + F],
                                start=(n == 0), stop=(n == 11))
                            n += 1
                    osb = spool.tile([128, 512], F32, name=f"osc{a}", tag="osb", bufs=2)
                    nc.vector.tensor_copy(osb[:], pt[:])
                    nc.sync.dma_start(out=trend_out.ap()[b, 128 * a : 128 * a + 128, :],
                                      in_=osb[:])

    nc.compile()
    return nc


def _host_prep(inputs):
    f32 = np.float32
    x = np.asarray(inputs["x"], f32)
    enc = np.asarray(inputs["enc_output"], f32)
    xt = np.ascontiguousarray(x.transpose(0, 2, 1))
    xtr = np.ascontiguousarray(xt[:, :, ::-1])
    ent = np.ascontiguousarray(enc.transpose(0, 2, 1))
    entr = np.ascontiguousarray(ent[:, :, ::-1])

    shared = {}
    for p in ("sa", "ca"):
        for nme in ("wq", "wk", "wv", "wo"):
            shared[f"{p}_{nme}"] = np.ascontiguousarray(np.asarray(inputs[f"{p}_{nme}"], f32))
    import ml_dtypes
    fc1 = np.asarray(inputs["fc1_w"], f32).reshape(NCH, 128, XP)       # (c, p, xp)
    fc2 = np.asarray(inputs["fc2_w"], f32).reshape(NXP, 128, E)        # (xc, p, e)
    fcw = np.zeros((128, 16384), ml_dtypes.bfloat16)
    fcw[:, :8192] = fc1.transpose(1, 0, 2).reshape(128, 8192).astype(ml_dtypes.bfloat16)
    fcw[:, 8192:] = fc2.transpose(1, 0, 2).reshape(128, 8192).astype(ml_dtypes.bfloat16)
    shared["fcw"] = fcw
    tw = np.asarray(inputs["trend_w"], f32)
    shared["wct"] = np.ascontiguousarray(tw.transpose(2, 1, 0).reshape(3 * E, F))

    def pc(v, nch=4):
        return np.ascontiguousarray(np.asarray(v, f32).reshape(nch, 128).T)

    shared["bias_pc"] = np.ascontiguousarray(np.concatenate([
        pc(inputs["sa_bq"]), pc(inputs["sa_bk"]), pc(inputs["sa_bo"]),
        pc(inputs["ca_bq"]), pc(inputs["ca_bk"]), pc(inputs["ca_bo"]),
        pc(inputs["fc2_b"]), pc(inputs["ln_g"]), pc(inputs["ln_b"]),
        pc(inputs["fc1_b"], 16),
    ], axis=1))
    shared["brow"] = np.ascontiguousarray(np.concatenate(
        [np.asarray(inputs["sa_bv"], f32), np.asarray(inputs["ca_bv"], f32)])[None, :])
    shared["ident"] = np.eye(128, dtype=f32)
    shared["identr"] = np.eye(128, dtype=f32)
    shared["jmat"] = np.ascontiguousarray(np.eye(128, dtype=f32)[::-1])
    shared["ones_div"] = np.full((128, 1), 1.0 / E, f32)
    shared["ones_row"] = np.ones((1, 128), f32)
    ramp = np.zeros(16, f32)
    ramp[:13] = np.arange(12, -1, -1)
    shared["rampl"] = np.tile(ramp, (128, 1))
    shared["ones12"] = np.ones((128, 12), f32)

    in_maps = []
    for core in range(NCORES):
        s = slice(core * BPC, (core + 1) * BPC)
        m = dict(shared)
        m["xt"] = np.ascontiguousarray(xt[s])
        m["xtr"] = np.ascontiguousarray(xtr[s])
        m["ent"] = np.ascontiguousarray(ent[s])
        m["entr"] = np.ascontiguousarray(entr[s])
        in_maps.append(m)
    return in_maps


_LAST = {}


def kernel(**inputs):
    from concourse.bass_utils import run_bass_kernel_spmd

    nc = _build()
    in_maps = _host_prep(inputs)
    res = run_bass_kernel_spmd(nc, in_maps, core_ids=list(range(NCORES)),
                               **_LAST.get("kwargs", {}))
    _LAST["res"] = res
    seasonal = np.concatenate([res.results[c]["seasonal"] for c in range(NCORES)], axis=0)
    trend = np.concatenate([res.results[c]["trend"] for c in range(NCORES)], axis=0)
    return seasonal, trend



# revision 12
# speedup vs baseline: 1.4400x; 1.4400x over previous
"""Trainium2 Bass kernel for nn_DecoderBlock (Autoformer decoder layer).

Data-parallel over batch: 16 batches -> 8 cores x 2 batches. Layout is
channel-major (E on partitions). The two per-core batches are software-
pipelined: emission order is hand-scheduled so the in-order PE queue never
waits on the softmax/DMA latency chains of the other batch.

Autocorrelation without FFTs:
  M[tau] = (1/E) sum_t <k_t, q_{(t+tau)%L}>
Key identity: the Gram element (s, s+tau) with s = 128*i + r lands at band
column j = r + tau for EVERY i-block, so all 8 i-blocks accumulate into ONE
PSUM band tile [128, 1152]; a single DRAM shear round-trip (row stride 1153
on readback) + one ones-matmul yields M.  Top-6 + softmax give a scattered
weight vector; the roll-aggregation
  agg[e,t] = sum_{s'} Vrev[s',e] * c2R[s'+t]
uses host-time-reversed V inputs and the reversed weight vector written
twice to DRAM, read back as a compact [128, 1920] sliding-window tile.

Trend: t1+t2+t3 accumulates in an SBUF bf16 tile (per-batch) written by the
decomposition stages; the circular conv reads it directly.

Precision: score matmuls in f32r (top-6 margins ~1e-4 in M units); FFN and
trend conv in bf16; everything else f32r.
"""
import functools
import numpy as np

NCORES = 8
BPC = 2
L = 1024
E = 512
XP = 2048
F = 512
KS = 25
NCH = 4
NT = 8
NXP = 16

PHASES = []


def _mark(nc, label):
    PHASES.append((label, nc.next_id()))


def _score_segments(i):
    """Column segments of the band [0, 1152) for k-block i, cut at the q wrap
    point jw = L - 128*i, each segment <=512 wide and >=256 where possible."""
    jw = L - 128 * i
    segs = []
    for lo, hi in ((0, jw), (jw, 1152)):
        w = hi - lo
        if w <= 0:
            continue
        n = (w + 511) // 512
        base = w // n
        rem = w - base * n
        st = lo
        for p in range(n):
            ln = base + (1 if p < rem else 0)
            segs.append((st, st + ln))
            st += ln
    return segs


@functools.lru_cache(maxsize=1)
def _build():
    import concourse.bacc as bacc
    import concourse.bass as bass
    import concourse.mybir as mybir
    from concourse.tile import TileContext

    F32 = mybir.dt.float32
    F32R = mybir.dt.float32r
    BF16 = mybir.dt.bfloat16
    AF = mybir.ActivationFunctionType
    OP = mybir.AluOpType

    nc = bacc.Bacc("TRN2", target_bir_lowering=False, debug=False, num_devices=NCORES)

    def din(name, shape, dtype=F32R):
        return nc.declare_dram_parameter(name, list(shape), dtype, isOutput=False)

    xt_in = din("xt", (BPC, E, L))
    xtr_in = din("xtr", (BPC, E, L))
    ent_in = din("ent", (BPC, E, L))
    entr_in = din("entr", (BPC, E, L))
    w_in = {}
    for p in ("sa", "ca"):
        for nme in ("wq", "wk", "wv", "wo"):
            w_in[f"{p}_{nme}"] = din(f"{p}_{nme}", (E, E))
    fcw_in = din("fcw", (128, 16384), BF16)   # packed fc1(c,2048)|fc2(xc,512), bf16
    wct_in = din("wct", (128, 12 * F), BF16)  # conv weights, bf16, [p, (j c), f]
    bias_pc_in = din("bias_pc", (128, 52), F32)
    ident_in = din("ident", (128, 128), F32)
    identr_in = din("identr", (128, 128), F32R)
    jmat_in = din("jmat", (128, 128), F32)
    ones_div_in = din("ones_div", (128, 1))
    ones_row_in = din("ones_row", (1, 128))
    rampl_in = din("rampl", (128, 16), F32)
    ones12_in = din("ones12", (128, 12), F32)

    seas_out = nc.declare_dram_parameter("seasonal", [BPC, L, E], F32, isOutput=True)
    trend_out = nc.declare_dram_parameter("trend", [BPC, L, F], F32, isOutput=True)

    a2d = {(b, p): nc.dram_tensor(f"a2d_{p}{b}", [128, 1152], F32R)
           for b in range(BPC) for p in ("sa", "ca")}
    c2rd = {(b, p): nc.dram_tensor(f"c2rd_{p}{b}", [2 * L], BF16)
            for b in range(BPC) for p in ("sa", "ca")}

    BQ = {"sa": 0, "ca": 12}
    BK = {"sa": 4, "ca": 16}
    BO = {"sa": 8, "ca": 20}
    FC2B, LNG, LNB, FC1B = 24, 28, 32, 36
    CW = 1026  # per-channel tacc row: [wrap | 1024 | wrap]

    P = {}  # current phase-scoped pool under key "a" (attention) / "l" (late)

    with TileContext(nc) as tc:
        with (
            tc.tile_pool(name="consts", bufs=1) as cpool,
            tc.tile_pool(name="wst", bufs=4) as wstp,
            tc.tile_pool(name="perm", bufs=1) as ppool,
            tc.tile_pool(name="str", bufs=1) as spool,
            tc.tile_pool(name="pp", bufs=2, space="PSUM") as pspool,
            tc.tile_pool(name="pa", bufs=1, space="PSUM") as papool,
            tc.tile_pool(name="pm", bufs=1, space="PSUM") as pmpool,
        ):
            ident = cpool.tile([128, 128], F32, name="ident")
            identr = cpool.tile([128, 128], F32R, name="identr")
            jmat = cpool.tile([128, 128], F32, name="jmat")
            ones_div = cpool.tile([128, 1], F32R, name="ones_div")
            ones_row = cpool.tile([1, 128], F32R, name="ones_row")
            rampl = cpool.tile([128, 16], F32, name="rampl")
            ones12 = cpool.tile([128, 12], F32, name="ones12")
            bias_pc = cpool.tile([128, 52], F32, name="bias_pc")
            for t, d in ((ident, ident_in), (identr, identr_in), (jmat, jmat_in),
                         (ones_div, ones_div_in), (ones_row, ones_row_in),
                         (rampl, rampl_in), (ones12, ones12_in),
                         (bias_pc, bias_pc_in)):
                nc.sync.dma_start(out=t[:], in_=d.ap())

            def ps_tile(name):
                return pspool.tile([128, 512], F32, name=name, tag="pp")

            y_t, x_t, xin_t, tacc_t = {}, {}, {}, {}
            for b in range(BPC):
                tacc_t[b] = ppool.tile([128, NCH * CW], BF16, name=f"tacc{b}",
                                       tag=f"tacc{b}", bufs=1)

            def load_xin(b, dram):
                t = P["a"].tile([128, NCH * L], F32R, name=f"xin{b}", tag="xin", bufs=1)
                nc.sync.dma_start(out=t[:].rearrange("p (c l) -> p c l", c=NCH),
                                  in_=dram.ap()[b].rearrange("(c p) l -> p c l", p=128))
                xin_t[b] = t
                return t

            # -------------------- building blocks --------------------
            def proj_el(out_t, src_t, w_dram, bcol, resid=None, resid_dram=None):
                """out[e_out,t] = sum_e w[e,e_out] src[e,t] + bias (+resid)."""
                for m in range(NCH):
                    wm = wstp.tile([128, 512], F32R, name=f"wm{m}", tag="wst")
                    nc.sync.dma_start(
                        out=wm[:].rearrange("p (c n) -> p c n", c=NCH),
                        in_=w_dram.ap().rearrange("(c p) n -> p c n", p=128)
                            [:, :, 128 * m : 128 * m + 128])
                    for h in range(2):
                        pt = ps_tile(f"pp{m}{h}")
                        first = True
                        if resid is not None:
                            nc.tensor.matmul(pt[:], identr[:],
                                             resid[:, m * L + 512 * h : m * L + 512 * h + 512],
                                             start=True, stop=False)
                            first = False
                        elif resid_dram is not None:
                            rs = wstp.tile([128, 512], F32R, name=f"rs{m}{h}", tag="wst")
                            nc.sync.dma_start(
                                out=rs[:],
                                in_=resid_dram.rearrange("(c p) l -> p c l", p=128)
                                    [:, m, 512 * h : 512 * h + 512])
                            nc.tensor.matmul(pt[:], identr[:], rs[:], start=True, stop=False)
                            first = False
                        for c in range(NCH):
                            nc.tensor.matmul(
                                pt[:], wm[:, c * 128 : c * 128 + 128],
                                src_t[:, c * L + 512 * h : c * L + 512 * h + 512],
                                start=(first and c == 0), stop=(c == NCH - 1))
                        dst = out_t[:, m * L + 512 * h : m * L + 512 * h + 512]
                        if (m + h) % 2 == 0:
                            nc.vector.tensor_scalar(dst, pt[:],
                                bias_pc[:, bcol + m : bcol + m + 1], None, op0=OP.add)
                        else:
                            nc.scalar.activation(dst, pt[:], AF.Identity,
                                bias=bias_pc[:, bcol + m : bcol + m + 1])

            def proj_rev(out_t, src_dram, w_dram):
                """time-reversed V in (L,E): out[s',e] = sum_e' xrev[e',s'] w[e',e].
                xrev streamed from DRAM in 128-wide t' chunks; bv folded into bo."""
                wv = []
                for c in range(NCH):
                    wc = wstp.tile([128, 512], F32R, name=f"wvc{c}", tag="wst")
                    nc.sync.dma_start(
                        out=wc[:],
                        in_=w_dram.ap().rearrange("(c p) n -> p c n", p=128)[:, c, :])
                    wv.append(wc)
                for j in range(NT):
                    rs = P["a"].tile([128, NCH * 128], F32R, name=f"rv{j}", tag="rv",
                                     bufs=3)
                    nc.sync.dma_start(
                        out=rs[:].rearrange("p (c l) -> p c l", c=NCH),
                        in_=src_dram.rearrange("(c p) l -> p c l", p=128)
                            [:, :, 128 * j : 128 * j + 128])
                    pt = ps_tile(f"pv{j}")
                    for c in range(NCH):
                        nc.tensor.matmul(pt[:], rs[:, c * 128 : c * 128 + 128],
                                         wv[c][:], start=(c == 0), stop=(c == NCH - 1))
                    if j % 2 == 0:
                        nc.vector.tensor_copy(out_t[:, j * E : (j + 1) * E], pt[:])
                    else:
                        nc.scalar.activation(out_t[:, j * E : (j + 1) * E], pt[:], AF.Copy)

            def scores(b, pfx, q_t, k_t):
                """Band-accumulated scores: pa[r, j] = sum_i G[128i+r, j+128i],
                then one DRAM shear round-trip."""
                pa = papool.tile([128, 1152], F32, name=f"pa{pfx}{b}", tag="pa")
                for i in range(NT):
                    segs = _score_segments(i)
                    for c in range(NCH):
                        lhs = k_t[:, c * L + 128 * i : c * L + 128 * i + 128]
                        for (j0, j1) in segs:
                            t0 = (j0 + 128 * i) % L
                            nc.tensor.matmul(
                                pa[:, j0:j1], lhs,
                                q_t[:, c * L + t0 : c * L + t0 + (j1 - j0)],
                                start=(i == 0 and c == 0),
                                stop=(i == NT - 1 and c == NCH - 1))
                bsb = P["a"].tile([128, 1152], F32R, name=f"bsb{pfx}{b}", tag="bsb", bufs=1)
                for h in range(3):
                    sl = slice(384 * h, 384 * h + 384)
                    if h % 2 == 0:
                        nc.scalar.activation(bsb[:, sl], pa[:, sl], AF.Copy)
                    else:
                        nc.vector.tensor_copy(bsb[:, sl], pa[:, sl])
                nc.sync.dma_start(out=a2d[(b, pfx)].ap(), in_=bsb[:])
                sch = P["a"].tile([128, L], F32R, name=f"sch{pfx}{b}", tag="sch", bufs=1)
                nc.sync.dma_start(
                    out=sch[:], in_=bass.AP(a2d[(b, pfx)], 0, [[1153, 128], [1, L]]))
                return sch

            def mps_reduce(b, pfx, sch):
                m_ps = pmpool.tile([1, L], F32, name=f"mps{pfx}{b}", tag="psbig")
                for h in range(2):
                    nc.tensor.matmul(m_ps[0:1, 512 * h : 512 * h + 512], ones_div[:],
                                     sch[:, 512 * h : 512 * h + 512],
                                     start=True, stop=True)
                return m_ps

            def softmax_c2(b, pfx, m_ps):
                """reverse M; scattered top-6 softmax -> c2R written twice to DRAM;
                compact cc tile loaded back (DVE DMA queue, behind its producers)."""
                m_row = spool.tile([1, L], F32, name=f"mrow{pfx}{b}", tag="mrow", bufs=1)
                nc.vector.tensor_copy(m_row[:], m_ps[:])
                colt_ps = pspool.tile([128, 8], F32, name=f"colt{pfx}{b}", tag="pp")
                for g in range(8):
                    nc.tensor.transpose(colt_ps[:, g : g + 1],
                                        m_row[0:1, 128 * g : 128 * g + 128], ident[0:1, 0:1])
                colt = spool.tile([128, 8], F32, name=f"coltsb{pfx}{b}", tag="coltsb", bufs=1)
                nc.vector.tensor_copy(colt[:], colt_ps[:])
                revt_ps = pspool.tile([128, 8], F32, name=f"revt{pfx}{b}", tag="pp")
                nc.tensor.matmul(revt_ps[:], jmat[:], colt[:], start=True, stop=True)
                revt = spool.tile([128, 8], F32, name=f"revtsb{pfx}{b}", tag="revtsb", bufs=1)
                nc.vector.tensor_copy(revt[:], revt_ps[:])
                mrev_ps = pmpool.tile([1, L], F32, name=f"mrevps{pfx}{b}", tag="psbig")
                for g in range(8):
                    nc.tensor.transpose(mrev_ps[0:1, 128 * (7 - g) : 128 * (7 - g) + 128],
                                        revt[:, g : g + 1], ident[:, :])
                mrev = spool.tile([1, L], F32, name=f"mrev{pfx}{b}", tag="mrev", bufs=1)
                nc.vector.tensor_copy(mrev[:], mrev_ps[:])

                top8 = spool.tile([1, 8], F32, name=f"top8{pfx}{b}", tag="top8", bufs=1)
                nc.vector.max(top8[:], mrev[:])
                negmax = spool.tile([1, 1], F32, name=f"ngm{pfx}{b}", tag="ngm", bufs=1)
                nc.vector.tensor_scalar(negmax[:], top8[:, 0:1], -1.0, None, op0=OP.mult)
                nc.vector.tensor_scalar(m_row[:], mrev[:], top8[:, 5:6], None, op0=OP.is_ge)
                nc.scalar.activation(mrev[:], mrev[:], AF.Exp, bias=negmax[:, 0:1])
                nc.vector.tensor_tensor(mrev[:], m_row[:], mrev[:], OP.mult)
                csum = spool.tile([1, 1], F32, name=f"csum{pfx}{b}", tag="csum", bufs=1)
                nc.vector.tensor_reduce(csum[:], mrev[:], axis=mybir.AxisListType.X, op=OP.add)
                crecip = spool.tile([1, 1], F32, name=f"crec{pfx}{b}", tag="crec", bufs=1)
                nc.vector.reciprocal(crecip[:], csum[:])
                c2r_sb = spool.tile([1, L], BF16, name=f"c2r{pfx}{b}", tag="c2r", bufs=1)
                nc.vector.tensor_scalar(c2r_sb[:], mrev[:], crecip[:, 0:1], None, op0=OP.mult)
                nc.scalar.dma_start(out=c2rd[(b, pfx)].ap()[0:L], in_=c2r_sb[:])
                nc.scalar.dma_start(out=c2rd[(b, pfx)].ap()[L:], in_=c2r_sb[:])
                cc = P["a"].tile([128, 1920], BF16, name=f"cc{pfx}{b}", tag="cc", bufs=1)
                nc.scalar.dma_start(out=cc[:], in_=bass.AP(c2rd[(b, pfx)], 0,
                                                           [[1, 128], [1, 1920]]))
                return cc

            def agg_block(b, pfx, vrev_t, cc, out_t):
                """agg[e,t] = sum_{s'} Vrev[s',e] c2R[s'+t] via compact cc tile."""
                for m in range(NCH):
                    for h in range(2):
                        pt = ps_tile(f"pag{m}{h}")
                        for j in range(NT):
                            nc.tensor.matmul(
                                pt[:], vrev_t[:, j * E + 128 * m : j * E + 128 * m + 128],
                                cc[:, 128 * j + 512 * h : 128 * j + 512 * h + 512],
                                start=(j == 0), stop=(j == NT - 1))
                        dsta = out_t[:, m * L + 512 * h : m * L + 512 * h + 512]
                        if (m + h) % 2 == 0:
                            nc.vector.tensor_copy(dsta, pt[:])
                        else:
                            nc.scalar.activation(dsta, pt[:], AF.Copy)

            def decomp(b, y_t_, xnext_t, stage):
                """xnext = y - movavg25(y); tacc (+)= movavg25(y) (bf16 SBUF).
                Scan chains (AP-scalar ops) on DVE; the heavy immediate-scalar
                ops (xnext, tacc) on gpsimd, xnext first."""
                tacc = tacc_t[b]
                ws_l = []
                for c in range(NCH):
                    y = lambda a, bb, c=c: y_t_[:, c * L + a : c * L + bb]
                    ws = spool.tile([128, L], F32, name=f"ws{c}", tag="ws", bufs=4)
                    cs25 = spool.tile([128, 25], F32, name=f"cs25{c}", tag=f"cs25{c}", bufs=1)
                    nc.vector.tensor_tensor_scan(cs25[:], y(0, 25), y(0, 25), 0.0,
                                                 op0=OP.add, op1=OP.bypass)
                    nc.vector.scalar_tensor_tensor(ws[:, 0:13], rampl[:, 0:13], y(0, 1),
                                                   cs25[:, 12:25], op0=OP.mult, op1=OP.add)
                    nc.vector.tensor_tensor_scan(ws[:, 13:1012], y(25, L), y(0, 999),
                                                 cs25[:, 24:25], op0=OP.add, op1=OP.subtract)
                    ylast = spool.tile([128, 12], F32, name=f"yl{c}", tag=f"yl{c}", bufs=1)
                    nc.vector.tensor_scalar(ylast[:], ones12[:], y(L - 1, L), None,
                                            op0=OP.mult)
                    nc.vector.tensor_tensor_scan(ws[:, 1012:1024], ylast[:], y(999, 1011),
                                                 ws[:, 1011:1012], op0=OP.add, op1=OP.subtract)
                    nc.vector.scalar_tensor_tensor(
                        xnext_t[:, c * L : (c + 1) * L], ws[:], -1.0 / KS, y(0, L),
                        op0=OP.mult, op1=OP.add)
                    ws_l.append(ws)
                for c in range(NCH):
                    ws = ws_l[c]
                    tsl = tacc[:, c * CW + 1 : c * CW + 1 + L]
                    if stage == 0:
                        nc.gpsimd.tensor_scalar(tsl, ws[:], 1.0 / KS, None, op0=OP.mult)
                    else:
                        wss = spool.tile([128, L], BF16, name=f"wss{c}", tag="wss", bufs=1)
                        nc.gpsimd.tensor_scalar(wss[:], ws[:], 1.0 / KS, None, op0=OP.mult)
                        nc.gpsimd.tensor_add(tsl, wss[:], tsl)
                    if stage == 2:
                        nc.gpsimd.tensor_copy(tacc[:, c * CW : c * CW + 1],
                                              tacc[:, c * CW + L : c * CW + L + 1])
                        nc.gpsimd.tensor_copy(tacc[:, c * CW + 1025 : c * CW + 1026],
                                              tacc[:, c * CW + 1 : c * CW + 2])

            def ffn_block(b, x3, fcw):
                x3bf = P["l"].tile([128, NCH * L], BF16, name=f"x3bf{b}", tag="x3bf", bufs=1)
                nc.vector.tensor_copy(x3bf[:], x3[:])
                y3 = ppool.tile([128, NCH * L], F32, name=f"y3{b}", tag="y", bufs=2)
                for half in range(2):
                    h_t = P["l"].tile([128, NXP * 512], BF16, name=f"h{b}{half}",
                                      tag="lbuf16", bufs=2)
                    for xc in range(NXP):
                        pt = ps_tile(f"ph{xc}")
                        for c in range(NCH):
                            nc.tensor.matmul(
                                pt[:], fcw[:, c * 2048 + 128 * xc : c * 2048 + 128 * xc + 128],
                                x3bf[:, c * L + 512 * half : c * L + 512 * half + 512],
                                start=(c == 0), stop=(c == NCH - 1))
                        nc.scalar.activation(h_t[:, xc * 512 : (xc + 1) * 512], pt[:],
                                             AF.Gelu, bias=bias_pc[:, FC1B + xc : FC1B + xc + 1])
                    for m in range(NCH):
                        pt = ps_tile(f"pf{m}")
                        for xc in range(NXP):
                            nc.tensor.matmul(
                                pt[:],
                                fcw[:, 8192 + xc * 512 + 128 * m : 8192 + xc * 512 + 128 * m + 128],
                                h_t[:, xc * 512 : (xc + 1) * 512],
                                start=(xc == 0), stop=(xc == NXP - 1))
                        sl = slice(m * L + 512 * half, m * L + 512 * half + 512)
                        nc.vector.scalar_tensor_tensor(
                            y3[:, sl], pt[:], bias_pc[:, FC2B + m : FC2B + m + 1],
                            x3[:, sl], op0=OP.add, op1=OP.add)
                return y3

            def ln_stats(b, x4):
                sq = P["l"].tile([128, NCH * L], F32R, name=f"sq{b}", tag="lbuf16", bufs=2)
                for c in range(NCH):
                    nc.scalar.activation(sq[:, c * L : (c + 1) * L],
                                         x4[:, c * L : (c + 1) * L], AF.Square)
                mu_ps = pmpool.tile([1, L], F32, name=f"mups{b}", tag="psbig")
                for h in range(2):
                    for c in range(NCH):
                        nc.tensor.matmul(mu_ps[0:1, 512 * h : 512 * h + 512], ones_div[:],
                                         x4[:, c * L + 512 * h : c * L + 512 * h + 512],
                                         start=(c == 0), stop=(c == NCH - 1))
                mu_r = spool.tile([1, L], F32, name=f"mur{b}", tag="mrow", bufs=1)
                nc.vector.tensor_copy(mu_r[:], mu_ps[:])
                ms_ps = pmpool.tile([1, L], F32, name=f"msps{b}", tag="psbig")
                for h in range(2):
                    for c in range(NCH):
                        nc.tensor.matmul(ms_ps[0:1, 512 * h : 512 * h + 512], ones_div[:],
                                         sq[:, c * L + 512 * h : c * L + 512 * h + 512],
                                         start=(c == 0), stop=(c == NCH - 1))
                var_r = spool.tile([1, L], F32, name=f"varr{b}", tag="mrev", bufs=1)
                nc.vector.tensor_tensor(var_r[:], mu_r[:], mu_r[:], OP.mult)
                nc.vector.scalar_tensor_tensor(var_r[:], ms_ps[:], 1e-5, var_r[:],
                                               op0=OP.add, op1=OP.subtract)
                nc.scalar.activation(var_r[:], var_r[:], AF.Sqrt)
                rows = spool.tile([1, L], F32R, name=f"rows{b}", tag="c2r", bufs=1)
                rows2 = spool.tile([1, L], F32R, name=f"rows2{b}", tag="rows2", bufs=1)
                with nc.allow_low_precision(reason="istd broadcast is f32r by design"):
                    nc.vector.reciprocal(rows[:], var_r[:])
                nc.vector.tensor_tensor(rows2[:], mu_r[:], rows[:], OP.mult)
                return rows, rows2

            def ln_apply(b, x4, rows, rows2):
                bc = P["l"].tile([128, 2 * L], F32, name=f"bc{b}", tag="lbuf16", bufs=2)
                for h in range(4):
                    bp = ps_tile(f"bc{h}")
                    src_row = rows if h < 2 else rows2
                    nc.tensor.matmul(bp[:], ones_row[:],
                                     src_row[0:1, 512 * (h % 2) : 512 * (h % 2) + 512],
                                     start=True, stop=True)
                    if h % 2 == 0:
                        nc.vector.tensor_copy(bc[:, 512 * h : 512 * h + 512], bp[:])
                    else:
                        nc.scalar.activation(bc[:, 512 * h : 512 * h + 512], bp[:], AF.Copy)
                seas = ppool.tile([128, NCH * L], F32, name=f"seas{b}", tag="y", bufs=2)
                accs = spool.tile([128, NCH], F32, name=f"accs{b}", tag="accs", bufs=2)
                for c in range(NCH):
                    t1 = spool.tile([128, L], F32, name=f"t1{c}", tag="ws", bufs=4)
                    nc.vector.tensor_tensor(t1[:], x4[:, c * L : (c + 1) * L],
                                            bc[:, 0:L], OP.mult)
                    nc.vector.tensor_tensor(t1[:], t1[:], bc[:, L:], OP.subtract)
                    nc.scalar.activation(seas[:, c * L : (c + 1) * L], t1[:], AF.Identity,
                                         bias=bias_pc[:, LNB + c : LNB + c + 1],
                                         scale=bias_pc[:, LNG + c : LNG + c + 1],
                                         accum_out=accs[:, c : c + 1])
                for c in range(NCH):
                    nc.vector.tensor_scalar(accs[:, c : c + 1], accs[:, c : c + 1],
                                            1.0 / L, None, op0=OP.mult)
                    nc.vector.tensor_scalar(seas[:, c * L : (c + 1) * L],
                                            seas[:, c * L : (c + 1) * L],
                                            accs[:, c : c + 1], None, op0=OP.subtract)
                return seas

            def seas_out_block(b, seas):
                for a in range(NT):
                    tp = ps_tile(f"tps{a}")
                    for c in range(NCH):
                        nc.tensor.transpose(tp[:, 128 * c : 128 * c + 128],
                                            seas[:, c * L + 128 * a : c * L + 128 * a + 128],
                                            ident[:, :])
                    osb = spool.tile([128, 512], F32, name=f"osb{a}", tag="osb", bufs=2)
                    if a % 2 == 0:
                        nc.vector.tensor_copy(osb[:], tp[:])
                    else:
                        nc.scalar.activation(osb[:], tp[:], AF.Copy)
                    nc.sync.dma_start(out=seas_out.ap()[b, 128 * a : 128 * a + 128, :],
                                      in_=osb[:])

            def trend_conv(b, wct):
                tacc = tacc_t[b]
                for a in range(NT):
                    pt = ps_tile(f"ptc{a}")
                    n = 0
                    for j in range(3):
                        for c in range(NCH):
                            nc.tensor.matmul(
                                pt[:],
                                tacc[:, c * CW + 128 * a + j : c * CW + 128 * a + j + 128],
                                wct[:, (j * NCH + c) * F : (j * NCH + c) * F + F],
                                start=(n == 0), stop=(n == 11))
                            n += 1
                    osb = spool.tile([128, 512], F32, name=f"osc{a}", tag="osb", bufs=2)
                    if a % 2 == 0:
                        nc.scalar.activation(osb[:], pt[:], AF.Copy)
                    else:
                        nc.vector.tensor_copy(osb[:], pt[:])
                    nc.sync.dma_start(out=trend_out.ap()[b, 128 * a : 128 * a + 128, :],
                                      in_=osb[:])

            # ============== attention phase: hand-scheduled emission ==============
            q_t, k_t, v_t, cc_t, sch_t = {}, {}, {}, {}, {}

            def attn_head(b, pfx):
                """proj q,k + band scores + shear round trip."""
                _mark(nc, f"{pfx}{b}.proj_qk")
                xq = x_t[b] if pfx == "ca" else xin_t[b]
                q = P["a"].tile([128, NCH * L], F32R, name=f"q{pfx}{b}", tag="q", bufs=1)
                k = P["a"].tile([128, NCH * L], F32R, name=f"k{pfx}{b}", tag="k", bufs=1)
                proj_el(q, xq, w_in[f"{pfx}_wq"], BQ[pfx])
                proj_el(k, xin_t[b], w_in[f"{pfx}_wk"], BK[pfx])
                _mark(nc, f"{pfx}{b}.scores")
                sch_t[b] = scores(b, pfx, q, k)

            def attn_mps(b, pfx):
                _mark(nc, f"{pfx}{b}.mps")
                mp = mps_reduce(b, pfx, sch_t[b])
                _mark(nc, f"{pfx}{b}.softmax")
                cc_t[b] = softmax_c2(b, pfx, mp)

            def attn_projv(b, pfx):
                _mark(nc, f"{pfx}{b}.projv")
                v = P["a"].tile([128, NT * E], BF16, name=f"v{pfx}{b}", tag="v", bufs=1)
                proj_rev(v, (xtr_in if pfx == "sa" else entr_in).ap()[b],
                         w_in[f"{pfx}_wv"])
                v_t[b] = v

            def attn_tail(b, pfx, stage, resid=None, resid_dram=None):
                """agg + output proj + decomp into x slot."""
                _mark(nc, f"{pfx}{b}.agg")
                ag = P["a"].tile([128, NCH * L], F32R, name=f"agg{pfx}{b}", tag="q", bufs=1)
                agg_block(b, pfx, v_t[b], cc_t[b], ag)
                _mark(nc, f"{pfx}{b}.projo")
                y = ppool.tile([128, NCH * L], F32, name=f"y{pfx}{b}", tag="y", bufs=2)
                proj_el(y, ag, w_in[f"{pfx}_wo"], BO[pfx], resid=resid,
                        resid_dram=resid_dram)
                _mark(nc, f"decomp{b}")
                xn = ppool.tile([128, NCH * L], F32R, name=f"x{pfx}{b}", tag="x", bufs=2)
                decomp(b, y, xn, stage)
                x_t[b] = xn

            with tc.tile_pool(name="attn", bufs=1) as atpool:
                P["a"] = atpool
                _mark(nc, "init")
                load_xin(0, xt_in)
                attn_head(0, "sa")
                load_xin(1, xt_in)
                attn_projv(0, "sa")
                attn_mps(0, "sa")
                attn_head(1, "sa")
                attn_tail(0, "sa", 0, resid_dram=xt_in.ap()[0])
                load_xin(0, ent_in)
                attn_projv(1, "sa")
                attn_mps(1, "sa")
                attn_head(0, "ca")                   # needs x_t[0] from decomp0
                attn_tail(1, "sa", 0, resid_dram=xt_in.ap()[1])
                load_xin(1, ent_in)
                attn_projv(0, "ca")
                attn_mps(0, "ca")
                attn_head(1, "ca")
                attn_tail(0, "ca", 1, resid=x_t[0])
                attn_projv(1, "ca")
                attn_mps(1, "ca")
                attn_tail(1, "ca", 1, resid=x_t[1])

            with tc.tile_pool(name="late", bufs=1) as ltpool:
                P["l"] = ltpool
                _mark(nc, "ffn_w")
                fcw = ltpool.tile([128, 16384], BF16, name="fcw", tag="bigw", bufs=1)
                nc.sync.dma_start(out=fcw[:], in_=fcw_in.ap())
                _mark(nc, "ffn0")
                y0 = ffn_block(0, x_t[0], fcw)
                _mark(nc, "decomp0")
                x40 = ppool.tile([128, NCH * L], F32R, name="x40", tag="x", bufs=2)
                decomp(0, y0, x40, 2)
                x_t[0] = x40
                _mark(nc, "ffn1")
                y1 = ffn_block(1, x_t[1], fcw)
                _mark(nc, "ln0a")
                r0, r20 = ln_stats(0, x40)
                _mark(nc, "decomp1")
                x41 = ppool.tile([128, NCH * L], F32R, name="x41", tag="x", bufs=2)
                decomp(1, y1, x41, 2)
                x_t[1] = x41
                _mark(nc, "trend_w")
                wct = ltpool.tile([128, 12 * F], BF16, name="wctt", tag="bigw", bufs=1)
                nc.sync.dma_start(out=wct[:], in_=wct_in.ap())
                _mark(nc, "trend0")
                trend_conv(0, wct)
                _mark(nc, "ln0b")
                seas0 = ln_apply(0, x40, r0, r20)
                _mark(nc, "ln1a")
                r1, r21 = ln_stats(1, x41)
                _mark(nc, "trend1")
                trend_conv(1, wct)
                _mark(nc, "ln0out")
                seas_out_block(0, seas0)
                _mark(nc, "ln1b")
                seas1 = ln_apply(1, x41, r1, r21)
                _mark(nc, "ln1out")
                seas_out_block(1, seas1)

    nc.compile()
    return nc


def _host_prep(inputs):
    f32 = np.float32
    import ml_dtypes
    x = np.asarray(inputs["x"], f32)
    enc = np.asarray(inputs["enc_output"], f32)
    xt = np.ascontiguousarray(x.transpose(0, 2, 1))
    xtr = np.ascontiguousarray(xt[:, :, ::-1])
    ent = np.ascontiguousarray(enc.transpose(0, 2, 1))
    entr = np.ascontiguousarray(ent[:, :, ::-1])

    shared = {}
    for p in ("sa", "ca"):
        for nme in ("wq", "wk", "wv", "wo"):
            shared[f"{p}_{nme}"] = np.ascontiguousarray(np.asarray(inputs[f"{p}_{nme}"], f32))
    fc1 = np.asarray(inputs["fc1_w"], f32).reshape(NCH, 128, XP)       # (c, p, xp)
    fc2 = np.asarray(inputs["fc2_w"], f32).reshape(NXP, 128, E)        # (xc, p, e)
    fcw = np.zeros((128, 16384), ml_dtypes.bfloat16)
    fcw[:, :8192] = fc1.transpose(1, 0, 2).reshape(128, 8192).astype(ml_dtypes.bfloat16)
    fcw[:, 8192:] = fc2.transpose(1, 0, 2).reshape(128, 8192).astype(ml_dtypes.bfloat16)
    shared["fcw"] = fcw
    tw = np.asarray(inputs["trend_w"], f32)
    # [p, (j c), f]: row (j*4+c)*128+p of the (3E, F) matrix = w[(j e)], e=128c+p
    wct = tw.transpose(2, 1, 0).reshape(3 * E, F)                      # [(j e), f]
    shared["wct"] = np.ascontiguousarray(
        wct.reshape(12, 128, F).transpose(1, 0, 2).reshape(128, 12 * F)
    ).astype(ml_dtypes.bfloat16)

    def pc(v, nch=4):
        return np.ascontiguousarray(np.asarray(v, f32).reshape(nch, 128).T)

    # fold bv into bo: out = Wo^T (agg + bv) + bo = Wo^T agg + (bv @ Wo + bo)
    sa_bo2 = np.asarray(inputs["sa_bv"], f32) @ np.asarray(inputs["sa_wo"], f32) \
        + np.asarray(inputs["sa_bo"], f32)
    ca_bo2 = np.asarray(inputs["ca_bv"], f32) @ np.asarray(inputs["ca_wo"], f32) \
        + np.asarray(inputs["ca_bo"], f32)
    shared["bias_pc"] = np.ascontiguousarray(np.concatenate([
        pc(inputs["sa_bq"]), pc(inputs["sa_bk"]), pc(sa_bo2),
        pc(inputs["ca_bq"]), pc(inputs["ca_bk"]), pc(ca_bo2),
        pc(inputs["fc2_b"]), pc(inputs["ln_g"]), pc(inputs["ln_b"]),
        pc(inputs["fc1_b"], 16),
    ], axis=1))
    shared["ident"] = np.eye(128, dtype=f32)
    shared["identr"] = np.eye(128, dtype=f32)
    shared["jmat"] = np.ascontiguousarray(np.eye(128, dtype=f32)[::-1])
    shared["ones_div"] = np.full((128, 1), 1.0 / E, f32)
    shared["ones_row"] = np.ones((1, 128), f32)
    ramp = np.zeros(16, f32)
    ramp[:13] = np.arange(12, -1, -1)
    shared["rampl"] = np.tile(ramp, (128, 1))
    shared["ones12"] = np.ones((128, 12), f32)

    in_maps = []
    for core in range(NCORES):
        s = slice(core * BPC, (core + 1) * BPC)
        m = dict(shared)
        m["xt"] = np.ascontiguousarray(xt[s])
        m["xtr"] = np.ascontiguousarray(xtr[s])
        m["ent"] = np.ascontiguousarray(ent[s])
        m["entr"] = np.ascontiguousarray(entr[s])
        in_maps.append(m)
    return in_maps


_LAST = {}


def kernel(**inputs):
    from concourse.bass_utils import run_bass_kernel_spmd

    nc = _build()
    in_maps = _host_prep(inputs)
    res = run_bass_kernel_spmd(nc, in_maps, core_ids=list(range(NCORES)),
                               **_LAST.get("kwargs", {}))
    _LAST["res"] = res
    seasonal = np.concatenate([res.results[c]["seasonal"] for c in range(NCORES)], axis=0)
    trend = np.concatenate([res.results[c]["trend"] for c in range(NCORES)], axis=0)
    return seasonal, trend


# revision 15
# speedup vs baseline: 1.4783x; 1.0266x over previous
"""Trainium2 Bass kernel for nn_DecoderBlock (Autoformer decoder layer).

Data-parallel over batch: 16 batches -> 8 cores x 2 batches. Layout is
channel-major (E on partitions). The two per-core batches are software-
pipelined: emission order is hand-scheduled so the in-order PE queue never
waits on the softmax/DMA latency chains of the other batch.

Autocorrelation without FFTs:
  M[tau] = (1/E) sum_t <k_t, q_{(t+tau)%L}>
Key identity: the Gram element (s, s+tau) with s = 128*i + r lands at band
column j = r + tau for EVERY i-block, so all 8 i-blocks accumulate into ONE
PSUM band tile [128, 1152]; a single DRAM shear round-trip (row stride 1153
on readback) + one ones-matmul yields M.  Top-6 + softmax give a scattered
weight vector; the roll-aggregation
  agg[e,t] = sum_{s'} Vrev[s',e] * c2R[s'+t]
uses host-time-reversed V inputs and the reversed weight vector written
twice to DRAM, read back as a compact [128, 1920] sliding-window tile.

Trend: t1+t2+t3 accumulates in an SBUF bf16 tile (per-batch) written by the
decomposition stages; the circular conv reads it directly.

Precision: score matmuls in f32r (top-6 margins ~1e-4 in M units); FFN and
trend conv in bf16; everything else f32r.
"""
import functools
import numpy as np

NCORES = 8
BPC = 2
L = 1024
E = 512
XP = 2048
F = 512
KS = 25
NCH = 4
NT = 8
NXP = 16

PHASES = []


def _mark(nc, label):
    PHASES.append((label, nc.next_id()))


def _score_segments(i):
    """Column segments of the band [0, 1152) for k-block i, cut at the q wrap
    point jw = L - 128*i, each segment <=512 wide and >=256 where possible."""
    jw = L - 128 * i
    segs = []
    for lo, hi in ((0, jw), (jw, 1152)):
        w = hi - lo
        if w <= 0:
            continue
        n = (w + 511) // 512
        base = w // n
        rem = w - base * n
        st = lo
        for p in range(n):
            ln = base + (1 if p < rem else 0)
            segs.append((st, st + ln))
            st += ln
    return segs


@functools.lru_cache(maxsize=1)
def _build():
    import concourse.bacc as bacc
    import concourse.bass as bass
    import concourse.mybir as mybir
    from concourse.tile import TileContext

    F32 = mybir.dt.float32
    F32R = mybir.dt.float32r
    BF16 = mybir.dt.bfloat16
    AF = mybir.ActivationFunctionType
    OP = mybir.AluOpType

    nc = bacc.Bacc("TRN2", target_bir_lowering=False, debug=False, num_devices=NCORES)

    def din(name, shape, dtype=F32R):
        return nc.declare_dram_parameter(name, list(shape), dtype, isOutput=False)

    xt_in = din("xt", (BPC, E, L))
    xtr_in = din("xtr", (BPC, E, L))
    ent_in = din("ent", (BPC, E, L))
    entr_in = din("entr", (BPC, E, L))
    w_in = {}
    for p in ("sa", "ca"):
        for nme in ("wq", "wk", "wv", "wo"):
            w_in[f"{p}_{nme}"] = din(f"{p}_{nme}", (E, E))
    fcw_in = din("fcw", (128, 16384), BF16)   # packed fc1(c,2048)|fc2(xc,512), bf16
    wct_in = din("wct", (128, 12 * F), BF16)  # conv weights, bf16, [p, (j c), f]
    consts_in = din("consts", (128, 336), F32)
    constr_in = din("constr", (128, 129), F32R)
    ones_row_in = din("ones_row", (1, 128))

    seas_out = nc.declare_dram_parameter("seasonal", [BPC, L, E], F32, isOutput=True)
    trend_out = nc.declare_dram_parameter("trend", [BPC, L, F], F32, isOutput=True)

    a2d = {(b, p): nc.dram_tensor(f"a2d_{p}{b}", [128, 1152], F32R)
           for b in range(BPC) for p in ("sa", "ca")}
    c2rd = {(b, p): nc.dram_tensor(f"c2rd_{p}{b}", [2 * L], BF16)
            for b in range(BPC) for p in ("sa", "ca")}

    BQ = {"sa": 0, "ca": 12}
    BK = {"sa": 4, "ca": 16}
    BO = {"sa": 8, "ca": 20}
    FC2B, LNG, LNB, FC1B = 24, 28, 32, 36
    CW = 1026  # per-channel tacc row: [wrap | 1024 | wrap]

    P = {}  # current phase-scoped pool under key "a" (attention) / "l" (late)

    with TileContext(nc) as tc:
        with (
            tc.tile_pool(name="consts", bufs=1) as cpool,
            tc.tile_pool(name="wst", bufs=4) as wstp,
            tc.tile_pool(name="perm", bufs=1) as ppool,
            tc.tile_pool(name="str", bufs=1) as spool,
            tc.tile_pool(name="pp", bufs=2, space="PSUM") as pspool,
            tc.tile_pool(name="pa", bufs=1, space="PSUM") as papool,
            tc.tile_pool(name="pm", bufs=1, space="PSUM") as pmpool,
        ):
            cpk = cpool.tile([128, 336], F32, name="consts")
            cpr = cpool.tile([128, 129], F32R, name="constr")
            ones_row = cpool.tile([1, 128], F32R, name="ones_row")
            nc.sync.dma_start(out=cpk[:], in_=consts_in.ap())
            nc.sync.dma_start(out=cpr[:], in_=constr_in.ap())
            nc.sync.dma_start(out=ones_row[:], in_=ones_row_in.ap())
            ident = cpk[:, 0:128]
            jmat = cpk[:, 128:256]
            rampl = cpk[:, 256:272]
            ones12 = cpk[:, 272:284]
            bias_pc = cpk[:, 284:336]
            identr = cpr[:, 0:128]
            ones_div = cpr[:, 128:129]

            def ps_tile(name):
                return pspool.tile([128, 512], F32, name=name, tag="pp")

            y_t, x_t, xin_t, tacc_t = {}, {}, {}, {}
            for b in range(BPC):
                tacc_t[b] = ppool.tile([128, NCH * CW], BF16, name=f"tacc{b}",
                                       tag=f"tacc{b}", bufs=1)

            def load_xin(b, dram, split=False):
                t = P["a"].tile([128, NCH * L], F32R, name=f"xin{b}", tag="xin", bufs=1)
                if split:
                    for c2 in range(NCH):
                        nc.sync.dma_start(
                            out=t[:].rearrange("p (c l) -> p c l", c=NCH)[:, c2:c2 + 1],
                            in_=dram.ap()[b].rearrange("(c p) l -> p c l", p=128)
                                [:, c2:c2 + 1])
                else:
                    nc.sync.dma_start(out=t[:].rearrange("p (c l) -> p c l", c=NCH),
                                      in_=dram.ap()[b].rearrange("(c p) l -> p c l", p=128))
                xin_t[b] = t
                return t

            # -------------------- building blocks --------------------
            def proj_el(out_t, src_t, w_dram, bcol, resid=None, resid_dram=None):
                """out[e_out,t] = sum_e w[e,e_out] src[e,t] + bias (+resid)."""
                for m in range(NCH):
                    wm = wstp.tile([128, 512], F32R, name=f"wm{m}", tag="wst")
                    nc.sync.dma_start(
                        out=wm[:].rearrange("p (c n) -> p c n", c=NCH),
                        in_=w_dram.ap().rearrange("(c p) n -> p c n", p=128)
                            [:, :, 128 * m : 128 * m + 128])
                    for h in range(2):
                        pt = ps_tile(f"pp{m}{h}")
                        first = True
                        if resid is not None:
                            nc.tensor.matmul(pt[:], identr,
                                             resid[:, m * L + 512 * h : m * L + 512 * h + 512],
                                             start=True, stop=False)
                            first = False
                        elif resid_dram is not None:
                            rs = wstp.tile([128, 512], F32R, name=f"rs{m}{h}", tag="wst")
                            nc.scalar.dma_start(
                                out=rs[:],
                                in_=resid_dram.rearrange("(c p) l -> p c l", p=128)
                                    [:, m, 512 * h : 512 * h + 512])
                            nc.tensor.matmul(pt[:], identr, rs[:], start=True, stop=False)
                            first = False
                        for c in range(NCH):
                            nc.tensor.matmul(
                                pt[:], wm[:, c * 128 : c * 128 + 128],
                                src_t[:, c * L + 512 * h : c * L + 512 * h + 512],
                                start=(first and c == 0), stop=(c == NCH - 1))
                        dst = out_t[:, m * L + 512 * h : m * L + 512 * h + 512]
                        if (m + h) % 2 == 0:
                            nc.vector.tensor_scalar(dst, pt[:],
                                bias_pc[:, bcol + m : bcol + m + 1], None, op0=OP.add)
                        else:
                            nc.scalar.activation(dst, pt[:], AF.Identity,
                                bias=bias_pc[:, bcol + m : bcol + m + 1])

            def proj_rev(out_t, src_dram, w_dram):
                """time-reversed V in (L,E): out[s',e] = sum_e' xrev[e',s'] w[e',e].
                xrev streamed from DRAM in 128-wide t' chunks; bv folded into bo."""
                wv = []
                for c in range(NCH):
                    wc = wstp.tile([128, 512], F32R, name=f"wvc{c}", tag="wst")
                    nc.sync.dma_start(
                        out=wc[:],
                        in_=w_dram.ap().rearrange("(c p) n -> p c n", p=128)[:, c, :])
                    wv.append(wc)
                for j in range(NT):
                    rs = P["a"].tile([128, NCH * 128], F32R, name=f"rv{j}", tag="rv",
                                     bufs=3)
                    nc.scalar.dma_start(
                        out=rs[:].rearrange("p (c l) -> p c l", c=NCH),
                        in_=src_dram.rearrange("(c p) l -> p c l", p=128)
                            [:, :, 128 * j : 128 * j + 128])
                    pt = ps_tile(f"pv{j}")
                    for c in range(NCH):
                        nc.tensor.matmul(pt[:], rs[:, c * 128 : c * 128 + 128],
                                         wv[c][:], start=(c == 0), stop=(c == NCH - 1))
                    if j % 2 == 0:
                        nc.vector.tensor_copy(out_t[:, j * E : (j + 1) * E], pt[:])
                    else:
                        nc.scalar.activation(out_t[:, j * E : (j + 1) * E], pt[:], AF.Copy)

            def scores(b, pfx, q_t, k_t):
                """Band-accumulated scores: pa[r, j] = sum_i G[128i+r, j+128i],
                then one DRAM shear round-trip."""
                pa = papool.tile([128, 1152], F32, name=f"pa{pfx}{b}", tag="pa")
                for i in range(NT):
                    segs = _score_segments(i)
                    for c in range(NCH):
                        lhs = k_t[:, c * L + 128 * i : c * L + 128 * i + 128]
                        for (j0, j1) in segs:
                            t0 = (j0 + 128 * i) % L
                            nc.tensor.matmul(
                                pa[:, j0:j1], lhs,
                                q_t[:, c * L + t0 : c * L + t0 + (j1 - j0)],
                                start=(i == 0 and c == 0),
                                stop=(i == NT - 1 and c == NCH - 1))
                bsb = P["a"].tile([128, 1152], F32R, name=f"bsb{pfx}{b}", tag="bsb", bufs=1)
                for h in range(3):
                    sl = slice(384 * h, 384 * h + 384)
                    if h % 2 == 0:
                        nc.scalar.activation(bsb[:, sl], pa[:, sl], AF.Copy)
                    else:
                        nc.vector.tensor_copy(bsb[:, sl], pa[:, sl])
                nc.sync.dma_start(out=a2d[(b, pfx)].ap(), in_=bsb[:])
                sch = P["a"].tile([128, L], F32R, name=f"sch{pfx}{b}", tag="sch", bufs=1)
                nc.sync.dma_start(
                    out=sch[:], in_=bass.AP(a2d[(b, pfx)], 0, [[1153, 128], [1, L]]))
                return sch

            def mps_reduce(b, pfx, sch):
                m_ps = pmpool.tile([1, L], F32, name=f"mps{pfx}{b}", tag="psbig")
                for h in range(2):
                    nc.tensor.matmul(m_ps[0:1, 512 * h : 512 * h + 512], ones_div,
                                     sch[:, 512 * h : 512 * h + 512],
                                     start=True, stop=True)
                return m_ps

            def softmax_c2(b, pfx, m_ps):
                """reverse M; scattered top-6 softmax -> c2R written twice to DRAM;
                compact cc tile loaded back (DVE DMA queue, behind its producers)."""
                m_row = spool.tile([1, L], F32, name=f"mrow{pfx}{b}", tag="mrow", bufs=1)
                nc.vector.tensor_copy(m_row[:], m_ps[:])
                colt_ps = pspool.tile([128, 8], F32, name=f"colt{pfx}{b}", tag="pp")
                for g in range(8):
                    nc.tensor.transpose(colt_ps[:, g : g + 1],
                                        m_row[0:1, 128 * g : 128 * g + 128], ident[0:1, 0:1])
                colt = spool.tile([128, 8], F32, name=f"coltsb{pfx}{b}", tag="coltsb", bufs=1)
                nc.vector.tensor_copy(colt[:], colt_ps[:])
                revt_ps = pspool.tile([128, 8], F32, name=f"revt{pfx}{b}", tag="pp")
                nc.tensor.matmul(revt_ps[:], jmat, colt[:], start=True, stop=True)
                revt = spool.tile([128, 8], F32, name=f"revtsb{pfx}{b}", tag="revtsb", bufs=1)
                nc.vector.tensor_copy(revt[:], revt_ps[:])
                mrev_ps = pmpool.tile([1, L], F32, name=f"mrevps{pfx}{b}", tag="psbig")
                for g in range(8):
                    nc.tensor.transpose(mrev_ps[0:1, 128 * (7 - g) : 128 * (7 - g) + 128],
                                        revt[:, g : g + 1], ident[:, :])
                mrev = spool.tile([1, L], F32, name=f"mrev{pfx}{b}", tag="mrev", bufs=1)
                nc.vector.tensor_copy(mrev[:], mrev_ps[:])

                top8 = spool.tile([1, 8], F32, name=f"top8{pfx}{b}", tag="top8", bufs=1)
                nc.vector.max(top8[:], mrev[:])
                negmax = spool.tile([1, 1], F32, name=f"ngm{pfx}{b}", tag="ngm", bufs=1)
                nc.vector.tensor_scalar(negmax[:], top8[:, 0:1], -1.0, None, op0=OP.mult)
                nc.vector.tensor_scalar(m_row[:], mrev[:], top8[:, 5:6], None, op0=OP.is_ge)
                nc.scalar.activation(mrev[:], mrev[:], AF.Exp, bias=negmax[:, 0:1])
                nc.vector.tensor_tensor(mrev[:], m_row[:], mrev[:], OP.mult)
                csum = spool.tile([1, 1], F32, name=f"csum{pfx}{b}", tag="csum", bufs=1)
                nc.vector.tensor_reduce(csum[:], mrev[:], axis=mybir.AxisListType.X, op=OP.add)
                crecip = spool.tile([1, 1], F32, name=f"crec{pfx}{b}", tag="crec", bufs=1)
                nc.vector.reciprocal(crecip[:], csum[:])
                c2r_sb = spool.tile([1, L], BF16, name=f"c2r{pfx}{b}", tag="c2r", bufs=1)
                nc.vector.tensor_scalar(c2r_sb[:], mrev[:], crecip[:, 0:1], None, op0=OP.mult)
                nc.scalar.dma_start(out=c2rd[(b, pfx)].ap()[0:L], in_=c2r_sb[:])
                nc.scalar.dma_start(out=c2rd[(b, pfx)].ap()[L:], in_=c2r_sb[:])
                cc = P["a"].tile([128, 1920], BF16, name=f"cc{pfx}{b}", tag="cc", bufs=1)
                nc.scalar.dma_start(out=cc[:], in_=bass.AP(c2rd[(b, pfx)], 0,
                                                           [[1, 128], [1, 1920]]))
                return cc

            def agg_block(b, pfx, vrev_t, cc, out_t):
                """agg[e,t] = sum_{s'} Vrev[s',e] c2R[s'+t] via compact cc tile."""
                for m in range(NCH):
                    for h in range(2):
                        pt = ps_tile(f"pag{m}{h}")
                        for j in range(NT):
                            nc.tensor.matmul(
                                pt[:], vrev_t[:, j * E + 128 * m : j * E + 128 * m + 128],
                                cc[:, 128 * j + 512 * h : 128 * j + 512 * h + 512],
                                start=(j == 0), stop=(j == NT - 1))
                        dsta = out_t[:, m * L + 512 * h : m * L + 512 * h + 512]
                        if (m + h) % 2 == 0:
                            nc.vector.tensor_copy(dsta, pt[:])
                        else:
                            nc.scalar.activation(dsta, pt[:], AF.Copy)

            def decomp(b, y_t_, xnext_t, stage):
                """xnext = y - movavg25(y); tacc (+)= movavg25(y) (bf16 SBUF).
                Scan chains (AP-scalar ops) on DVE; the heavy immediate-scalar
                ops (xnext, tacc) on gpsimd, xnext first."""
                tacc = tacc_t[b]
                ws_l = []
                for c in range(NCH):
                    y = lambda a, bb, c=c: y_t_[:, c * L + a : c * L + bb]
                    ws = spool.tile([128, L], F32, name=f"ws{c}", tag="ws", bufs=4)
                    cs25 = spool.tile([128, 25], F32, name=f"cs25{c}", tag=f"cs25{c}", bufs=1)
                    nc.vector.tensor_tensor_scan(cs25[:], y(0, 25), y(0, 25), 0.0,
                                                 op0=OP.add, op1=OP.bypass)
                    nc.vector.scalar_tensor_tensor(ws[:, 0:13], rampl[:, 0:13], y(0, 1),
                                                   cs25[:, 12:25], op0=OP.mult, op1=OP.add)
                    nc.vector.tensor_tensor_scan(ws[:, 13:1012], y(25, L), y(0, 999),
                                                 cs25[:, 24:25], op0=OP.add, op1=OP.subtract)
                    ylast = spool.tile([128, 12], F32, name=f"yl{c}", tag=f"yl{c}", bufs=1)
                    nc.vector.tensor_scalar(ylast[:], ones12, y(L - 1, L), None,
                                            op0=OP.mult)
                    nc.vector.tensor_tensor_scan(ws[:, 1012:1024], ylast[:], y(999, 1011),
                                                 ws[:, 1011:1012], op0=OP.add, op1=OP.subtract)
                    nc.vector.scalar_tensor_tensor(
                        xnext_t[:, c * L : (c + 1) * L], ws[:], -1.0 / KS, y(0, L),
                        op0=OP.mult, op1=OP.add)
                    ws_l.append(ws)
                for c in range(NCH):
                    ws = ws_l[c]
                    tsl = tacc[:, c * CW + 1 : c * CW + 1 + L]
                    if stage == 0:
                        nc.gpsimd.tensor_scalar(tsl, ws[:], 1.0 / KS, None, op0=OP.mult)
                    else:
                        wss = spool.tile([128, L], BF16, name=f"wss{c}", tag="wss", bufs=1)
                        nc.gpsimd.tensor_scalar(wss[:], ws[:], 1.0 / KS, None, op0=OP.mult)
                        nc.gpsimd.tensor_add(tsl, wss[:], tsl)
                    if stage == 2:
                        nc.gpsimd.tensor_copy(tacc[:, c * CW : c * CW + 1],
                                              tacc[:, c * CW + L : c * CW + L + 1])
                        nc.gpsimd.tensor_copy(tacc[:, c * CW + 1025 : c * CW + 1026],
                                              tacc[:, c * CW + 1 : c * CW + 2])

            def ffn_block(b, x3, fcw):
                x3bf = P["l"].tile([128, NCH * L], BF16, name=f"x3bf{b}", tag="x3bf", bufs=1)
                nc.vector.tensor_copy(x3bf[:], x3[:])
                y3 = ppool.tile([128, NCH * L], F32, name=f"y3{b}", tag="y", bufs=2)
                for half in range(2):
                    h_t = P["l"].tile([128, NXP * 512], BF16, name=f"h{b}{half}",
                                      tag="lbuf16", bufs=2)
                    for xc in range(NXP):
                        pt = ps_tile(f"ph{xc}")
                        for c in range(NCH):
                            nc.tensor.matmul(
                                pt[:], fcw[:, c * 2048 + 128 * xc : c * 2048 + 128 * xc + 128],
                                x3bf[:, c * L + 512 * half : c * L + 512 * half + 512],
                                start=(c == 0), stop=(c == NCH - 1))
                        nc.scalar.activation(h_t[:, xc * 512 : (xc + 1) * 512], pt[:],
                                             AF.Gelu, bias=bias_pc[:, FC1B + xc : FC1B + xc + 1])
                    for m in range(NCH):
                        pt = ps_tile(f"pf{m}")
                        for xc in range(NXP):
                            nc.tensor.matmul(
                                pt[:],
                                fcw[:, 8192 + xc * 512 + 128 * m : 8192 + xc * 512 + 128 * m + 128],
                                h_t[:, xc * 512 : (xc + 1) * 512],
                                start=(xc == 0), stop=(xc == NXP - 1))
                        sl = slice(m * L + 512 * half, m * L + 512 * half + 512)
                        nc.vector.scalar_tensor_tensor(
                            y3[:, sl], pt[:], bias_pc[:, FC2B + m : FC2B + m + 1],
                            x3[:, sl], op0=OP.add, op1=OP.add)
                return y3

            def ln_stats(b, x4):
                sq = P["l"].tile([128, NCH * L], F32R, name=f"sq{b}", tag="lbuf16", bufs=2)
                for c in range(NCH):
                    nc.scalar.activation(sq[:, c * L : (c + 1) * L],
                                         x4[:, c * L : (c + 1) * L], AF.Square)
                mu_ps = pmpool.tile([1, L], F32, name=f"mups{b}", tag="psbig")
                for h in range(2):
                    for c in range(NCH):
                        nc.tensor.matmul(mu_ps[0:1, 512 * h : 512 * h + 512], ones_div,
                                         x4[:, c * L + 512 * h : c * L + 512 * h + 512],
                                         start=(c == 0), stop=(c == NCH - 1))
                mu_r = spool.tile([1, L], F32, name=f"mur{b}", tag="mrow", bufs=1)
                nc.vector.tensor_copy(mu_r[:], mu_ps[:])
                ms_ps = pmpool.tile([1, L], F32, name=f"msps{b}", tag="psbig")
                for h in range(2):
                    for c in range(NCH):
                        nc.tensor.matmul(ms_ps[0:1, 512 * h : 512 * h + 512], ones_div,
                                         sq[:, c * L + 512 * h : c * L + 512 * h + 512],
                                         start=(c == 0), stop=(c == NCH - 1))
                var_r = spool.tile([1, L], F32, name=f"varr{b}", tag="mrev", bufs=1)
                nc.vector.tensor_tensor(var_r[:], mu_r[:], mu_r[:], OP.mult)
                nc.vector.scalar_tensor_tensor(var_r[:], ms_ps[:], 1e-5, var_r[:],
                                               op0=OP.add, op1=OP.subtract)
                nc.scalar.activation(var_r[:], var_r[:], AF.Sqrt)
                rows = spool.tile([1, L], F32R, name=f"rows{b}", tag="c2r", bufs=1)
                rows2 = spool.tile([1, L], F32R, name=f"rows2{b}", tag="rows2", bufs=1)
                with nc.allow_low_precision(reason="istd broadcast is f32r by design"):
                    nc.vector.reciprocal(rows[:], var_r[:])
                nc.vector.tensor_tensor(rows2[:], mu_r[:], rows[:], OP.mult)
                return rows, rows2

            def ln_apply(b, x4, rows, rows2):
                bc = P["l"].tile([128, 2 * L], F32, name=f"bc{b}", tag="lbuf16", bufs=2)
                for h in range(4):
                    bp = ps_tile(f"bc{h}")
                    src_row = rows if h < 2 else rows2
                    nc.tensor.matmul(bp[:], ones_row[:],
                                     src_row[0:1, 512 * (h % 2) : 512 * (h % 2) + 512],
                                     start=True, stop=True)
                    if h % 2 == 0:
                        nc.vector.tensor_copy(bc[:, 512 * h : 512 * h + 512], bp[:])
                    else:
                        nc.scalar.activation(bc[:, 512 * h : 512 * h + 512], bp[:], AF.Copy)
                seas = ppool.tile([128, NCH * L], F32, name=f"seas{b}", tag="y", bufs=2)
                accs = spool.tile([128, NCH], F32, name=f"accs{b}", tag="accs", bufs=2)
                for c in range(NCH):
                    t1 = spool.tile([128, L], F32, name=f"t1{c}", tag="ws", bufs=4)
                    nc.vector.tensor_tensor(t1[:], x4[:, c * L : (c + 1) * L],
                                            bc[:, 0:L], OP.mult)
                    nc.vector.tensor_tensor(t1[:], t1[:], bc[:, L:], OP.subtract)
                    nc.scalar.activation(seas[:, c * L : (c + 1) * L], t1[:], AF.Identity,
                                         bias=bias_pc[:, LNB + c : LNB + c + 1],
                                         scale=bias_pc[:, LNG + c : LNG + c + 1],
                                         accum_out=accs[:, c : c + 1])
                for c in range(NCH):
                    nc.vector.tensor_scalar(accs[:, c : c + 1], accs[:, c : c + 1],
                                            1.0 / L, None, op0=OP.mult)
                    nc.vector.tensor_scalar(seas[:, c * L : (c + 1) * L],
                                            seas[:, c * L : (c + 1) * L],
                                            accs[:, c : c + 1], None, op0=OP.subtract)
                return seas

            def seas_out_block(b, seas):
                for a in range(NT):
                    tp = ps_tile(f"tps{a}")
                    for c in range(NCH):
                        nc.tensor.transpose(tp[:, 128 * c : 128 * c + 128],
                                            seas[:, c * L + 128 * a : c * L + 128 * a + 128],
                                            ident[:, :])
                    osb = spool.tile([128, 512], F32, name=f"osb{a}", tag="osb", bufs=2)
                    if a % 2 == 0:
                        nc.vector.tensor_copy(osb[:], tp[:])
                    else:
                        nc.scalar.activation(osb[:], tp[:], AF.Copy)
                    nc.sync.dma_start(out=seas_out.ap()[b, 128 * a : 128 * a + 128, :],
                                      in_=osb[:])

            def trend_conv(b, wct):
                tacc = tacc_t[b]
                for a in range(NT):
                    pt = ps_tile(f"ptc{a}")
                    n = 0
                    for j in range(3):
                        for c in range(NCH):
                            nc.tensor.matmul(
                                pt[:],
                                tacc[:, c * CW + 128 * a + j : c * CW + 128 * a + j + 128],
                                wct[:, (j * NCH + c) * F : (j * NCH + c) * F + F],
                                start=(n == 0), stop=(n == 11))
                            n += 1
                    osb = spool.tile([128, 512], F32, name=f"osc{a}", tag="osb", bufs=2)
                    if a % 2 == 0:
                        nc.scalar.activation(osb[:], pt[:], AF.Copy)
                    else:
                        nc.vector.tensor_copy(osb[:], pt[:])
                    nc.sync.dma_start(out=trend_out.ap()[b, 128 * a : 128 * a + 128, :],
                                      in_=osb[:])

            # ============== attention phase: hand-scheduled emission ==============
            q_t, k_t, v_t, cc_t, sch_t = {}, {}, {}, {}, {}

            def attn_head(b, pfx):
                """proj q,k + band scores + shear round trip."""
                _mark(nc, f"{pfx}{b}.proj_qk")
                xq = x_t[b] if pfx == "ca" else xin_t[b]
                q = P["a"].tile([128, NCH * L], F32R, name=f"q{pfx}{b}", tag="q", bufs=1)
                k = P["a"].tile([128, NCH * L], F32R, name=f"k{pfx}{b}", tag="k", bufs=1)
                proj_el(q, xq, w_in[f"{pfx}_wq"], BQ[pfx])
                proj_el(k, xin_t[b], w_in[f"{pfx}_wk"], BK[pfx])
                _mark(nc, f"{pfx}{b}.scores")
                sch_t[b] = scores(b, pfx, q, k)

            def attn_mps(b, pfx):
                _mark(nc, f"{pfx}{b}.mps")
                mp = mps_reduce(b, pfx, sch_t[b])
                _mark(nc, f"{pfx}{b}.softmax")
                cc_t[b] = softmax_c2(b, pfx, mp)

            def attn_projv(b, pfx):
                _mark(nc, f"{pfx}{b}.projv")
                v = P["a"].tile([128, NT * E], BF16, name=f"v{pfx}{b}", tag="v", bufs=1)
                proj_rev(v, (xtr_in if pfx == "sa" else entr_in).ap()[b],
                         w_in[f"{pfx}_wv"])
                v_t[b] = v

            ag_t = {}

            def attn_agg(b, pfx):
                _mark(nc, f"{pfx}{b}.agg")
                ag = P["a"].tile([128, NCH * L], F32R, name=f"agg{pfx}{b}", tag="q", bufs=1)
                agg_block(b, pfx, v_t[b], cc_t[b], ag)
                ag_t[b] = ag

            def attn_projo(b, pfx, stage, resid=None, resid_dram=None):
                _mark(nc, f"{pfx}{b}.projo")
                y = ppool.tile([128, NCH * L], F32, name=f"y{pfx}{b}", tag="y", bufs=2)
                proj_el(y, ag_t[b], w_in[f"{pfx}_wo"], BO[pfx], resid=resid,
                        resid_dram=resid_dram)
                _mark(nc, f"decomp{b}")
                xn = ppool.tile([128, NCH * L], F32R, name=f"x{pfx}{b}", tag="x", bufs=2)
                decomp(b, y, xn, stage)
                x_t[b] = xn

            def attn_tail(b, pfx, stage, resid=None, resid_dram=None):
                attn_agg(b, pfx)
                attn_projo(b, pfx, stage, resid=resid, resid_dram=resid_dram)

            with tc.tile_pool(name="attn", bufs=1) as atpool:
                P["a"] = atpool
                _mark(nc, "init")
                load_xin(0, xt_in, split=True)
                attn_head(0, "sa")
                load_xin(1, xt_in)
                attn_projv(0, "sa")
                attn_mps(0, "sa")
                attn_head(1, "sa")
                attn_tail(0, "sa", 0, resid_dram=xt_in.ap()[0])
                load_xin(0, ent_in)
                attn_projv(1, "sa")
                attn_mps(1, "sa")
                attn_head(0, "ca")                   # needs x_t[0] from decomp0
                attn_tail(1, "sa", 0, resid_dram=xt_in.ap()[1])
                load_xin(1, ent_in)
                attn_projv(0, "ca")
                attn_mps(0, "ca")
                attn_head(1, "ca")
                attn_tail(0, "ca", 1, resid=x_t[0])
                attn_projv(1, "ca")
                attn_mps(1, "ca")
                attn_tail(1, "ca", 1, resid=x_t[1])

            with tc.tile_pool(name="late", bufs=1) as ltpool:
                P["l"] = ltpool
                _mark(nc, "ffn_w")
                fcw = ltpool.tile([128, 16384], BF16, name="fcw", tag="bigw", bufs=1)
                nc.gpsimd.dma_start(out=fcw[:], in_=fcw_in.ap())
                _mark(nc, "ffn0")
                y0 = ffn_block(0, x_t[0], fcw)
                _mark(nc, "decomp0")
                x40 = ppool.tile([128, NCH * L], F32R, name="x40", tag="x", bufs=2)
                decomp(0, y0, x40, 2)
                x_t[0] = x40
                _mark(nc, "ffn1")
                y1 = ffn_block(1, x_t[1], fcw)
                _mark(nc, "ln0a")
                r0, r20 = ln_stats(0, x40)
                _mark(nc, "decomp1")
                x41 = ppool.tile([128, NCH * L], F32R, name="x41", tag="x", bufs=2)
                decomp(1, y1, x41, 2)
                x_t[1] = x41
                _mark(nc, "trend_w")
                wct = ltpool.tile([128, 12 * F], BF16, name="wctt", tag="bigw", bufs=1)
                nc.sync.dma_start(out=wct[:], in_=wct_in.ap())
                _mark(nc, "trend0")
                trend_conv(0, wct)
                _mark(nc, "ln0b")
                seas0 = ln_apply(0, x40, r0, r20)
                _mark(nc, "ln1a")
                r1, r21 = ln_stats(1, x41)
                _mark(nc, "ln0out")
                seas_out_block(0, seas0)
                _mark(nc, "ln1b")
                seas1 = ln_apply(1, x41, r1, r21)
                _mark(nc, "trend1")
                trend_conv(1, wct)
                _mark(nc, "ln1out")
                seas_out_block(1, seas1)

    nc.compile()
    return nc


def _host_prep(inputs):
    f32 = np.float32
    import ml_dtypes
    x = np.asarray(inputs["x"], f32)
    enc = np.asarray(inputs["enc_output"], f32)
    xt = np.ascontiguousarray(x.transpose(0, 2, 1))
    xtr = np.ascontiguousarray(xt[:, :, ::-1])
    ent = np.ascontiguousarray(enc.transpose(0, 2, 1))
    entr = np.ascontiguousarray(ent[:, :, ::-1])

    shared = {}
    for p in ("sa", "ca"):
        for nme in ("wq", "wk", "wv", "wo"):
            shared[f"{p}_{nme}"] = np.ascontiguousarray(np.asarray(inputs[f"{p}_{nme}"], f32))
    fc1 = np.asarray(inputs["fc1_w"], f32).reshape(NCH, 128, XP)       # (c, p, xp)
    fc2 = np.asarray(inputs["fc2_w"], f32).reshape(NXP, 128, E)        # (xc, p, e)
    fcw = np.zeros((128, 16384), ml_dtypes.bfloat16)
    fcw[:, :8192] = fc1.transpose(1, 0, 2).reshape(128, 8192).astype(ml_dtypes.bfloat16)
    fcw[:, 8192:] = fc2.transpose(1, 0, 2).reshape(128, 8192).astype(ml_dtypes.bfloat16)
    shared["fcw"] = fcw
    tw = np.asarray(inputs["trend_w"], f32)
    # [p, (j c), f]: row (j*4+c)*128+p of the (3E, F) matrix = w[(j e)], e=128c+p
    wct = tw.transpose(2, 1, 0).reshape(3 * E, F)                      # [(j e), f]
    shared["wct"] = np.ascontiguousarray(
        wct.reshape(12, 128, F).transpose(1, 0, 2).reshape(128, 12 * F)
    ).astype(ml_dtypes.bfloat16)

    def pc(v, nch=4):
        return np.ascontiguousarray(np.asarray(v, f32).reshape(nch, 128).T)

    # fold bv into bo: out = Wo^T (agg + bv) + bo = Wo^T agg + (bv @ Wo + bo)
    sa_bo2 = np.asarray(inputs["sa_bv"], f32) @ np.asarray(inputs["sa_wo"], f32) \
        + np.asarray(inputs["sa_bo"], f32)
    ca_bo2 = np.asarray(inputs["ca_bv"], f32) @ np.asarray(inputs["ca_wo"], f32) \
        + np.asarray(inputs["ca_bo"], f32)
    shared["bias_pc"] = np.ascontiguousarray(np.concatenate([
        pc(inputs["sa_bq"]), pc(inputs["sa_bk"]), pc(sa_bo2),
        pc(inputs["ca_bq"]), pc(inputs["ca_bk"]), pc(ca_bo2),
        pc(inputs["fc2_b"]), pc(inputs["ln_g"]), pc(inputs["ln_b"]),
        pc(inputs["fc1_b"], 16),
    ], axis=1))
    ramp = np.zeros(16, f32)
    ramp[:13] = np.arange(12, -1, -1)
    shared["consts"] = np.ascontiguousarray(np.concatenate([
        np.eye(128, dtype=f32),                       # ident 0:128
        np.eye(128, dtype=f32)[::-1],                 # jmat 128:256
        np.tile(ramp, (128, 1)),                      # rampl 256:272
        np.ones((128, 12), f32),                      # ones12 272:284
        shared.pop("bias_pc"),                        # bias_pc 284:336
    ], axis=1))
    shared["constr"] = np.ascontiguousarray(np.concatenate([
        np.eye(128, dtype=f32),                       # identr 0:128
        np.full((128, 1), 1.0 / E, f32),              # ones_div 128:129
    ], axis=1))
    shared["ones_row"] = np.ones((1, 128), f32)

    in_maps = []
    for core in range(NCORES):
        s = slice(core * BPC, (core + 1) * BPC)
        m = dict(shared)
        m["xt"] = np.ascontiguousarray(xt[s])
        m["xtr"] = np.ascontiguousarray(xtr[s])
        m["ent"] = np.ascontiguousarray(ent[s])
        m["entr"] = np.ascontiguousarray(entr[s])
        in_maps.append(m)
    return in_maps


_LAST = {}


def kernel(**inputs):
    from concourse.bass_utils import run_bass_kernel_spmd

    nc = _build()
    in_maps = _host_prep(inputs)
    res = run_bass_kernel_spmd(nc, in_maps, core_ids=list(range(NCORES)),
                               **_LAST.get("kwargs", {}))
    _LAST["res"] = res
    seasonal = np.concatenate([res.results[c]["seasonal"] for c in range(NCORES)], axis=0)
    trend = np.concatenate([res.results[c]["trend"] for c in range(NCORES)], axis=0)
    return seasonal, trend


# revision 16
# speedup vs baseline: 1.5938x; 1.0781x over previous
"""Trainium2 Bass kernel for nn_DecoderBlock (Autoformer decoder layer).

Data-parallel over batch: 16 batches -> 8 cores x 2 batches. Layout is
channel-major (E on partitions). The two per-core batches are software-
pipelined: emission order is hand-scheduled so the in-order PE queue never
waits on the softmax/DMA latency chains of the other batch.

Autocorrelation without FFTs:
  M[tau] = (1/E) sum_t <k_t, q_{(t+tau)%L}>
Key identity: the Gram element (s, s+tau) with s = 128*i + r lands at band
column j = r + tau for EVERY i-block, so all 8 i-blocks accumulate into ONE
PSUM band tile [128, 1152]; a single DRAM shear round-trip (row stride 1153
on readback) + one ones-matmul yields M.  Top-6 + softmax give a scattered
weight vector; the roll-aggregation
  agg[e,t] = sum_{s'} Vrev[s',e] * c2R[s'+t]
uses host-time-reversed V inputs and the reversed weight vector written
twice to DRAM, read back as a compact [128, 1920] sliding-window tile.

Trend: t1+t2+t3 accumulates in an SBUF bf16 tile (per-batch) written by the
decomposition stages; the circular conv reads it directly.

Precision: score matmuls in f32r (top-6 margins ~1e-4 in M units); FFN and
trend conv in bf16; everything else f32r.
"""
import functools
import numpy as np

NCORES = 8
BPC = 2
L = 1024
E = 512
XP = 2048
F = 512
KS = 25
NCH = 4
NT = 8
NXP = 16

PHASES = []


def _mark(nc, label):
    PHASES.append((label, nc.next_id()))


def _score_segments(i):
    """Column segments of the band [0, 1152) for k-block i, cut at the q wrap
    point jw = L - 128*i, each segment <=512 wide and >=256 where possible."""
    jw = L - 128 * i
    segs = []
    for lo, hi in ((0, jw), (jw, 1152)):
        w = hi - lo
        if w <= 0:
            continue
        n = (w + 511) // 512
        base = w // n
        rem = w - base * n
        st = lo
        for p in range(n):
            ln = base + (1 if p < rem else 0)
            segs.append((st, st + ln))
            st += ln
    return segs


@functools.lru_cache(maxsize=1)
def _build():
    import concourse.bacc as bacc
    import concourse.bass as bass
    import concourse.mybir as mybir
    from concourse.tile import TileContext

    F32 = mybir.dt.float32
    F32R = mybir.dt.float32r
    BF16 = mybir.dt.bfloat16
    AF = mybir.ActivationFunctionType
    OP = mybir.AluOpType

    nc = bacc.Bacc("TRN2", target_bir_lowering=False, debug=False, num_devices=NCORES)

    def din(name, shape, dtype=F32R):
        return nc.declare_dram_parameter(name, list(shape), dtype, isOutput=False)

    xt_in = din("xt", (BPC, E, L))
    xtr_in = din("xtr", (BPC, E, L))
    ent_in = din("ent", (BPC, E, L))
    entr_in = din("entr", (BPC, E, L))
    w_in = {}
    for p in ("sa", "ca"):
        for nme in ("wq", "wk", "wv", "wo"):
            w_in[f"{p}_{nme}"] = din(f"{p}_{nme}", (E, E))
    fcw_in = din("fcw", (128, 16384), BF16)   # packed fc1(c,2048)|fc2(xc,512), bf16
    wct_in = din("wct", (128, 12 * F), BF16)  # conv weights, bf16, [p, (j c), f]
    consts_in = din("consts", (128, 336), F32)
    constr_in = din("constr", (128, 129), F32R)
    ones_row_in = din("ones_row", (1, 128))

    seas_out = nc.declare_dram_parameter("seasonal", [BPC, L, E], F32, isOutput=True)
    trend_out = nc.declare_dram_parameter("trend", [BPC, L, F], F32, isOutput=True)

    a2d = {(b, p): nc.dram_tensor(f"a2d_{p}{b}", [128, 1152], F32R)
           for b in range(BPC) for p in ("sa", "ca")}
    c2rd = {(b, p): nc.dram_tensor(f"c2rd_{p}{b}", [2 * L], BF16)
            for b in range(BPC) for p in ("sa", "ca")}

    BQ = {"sa": 0, "ca": 12}
    BK = {"sa": 4, "ca": 16}
    BO = {"sa": 8, "ca": 20}
    FC2B, LNG, LNB, FC1B = 24, 28, 32, 36
    CW = 1026  # per-channel tacc row: [wrap | 1024 | wrap]

    P = {}  # current phase-scoped pool under key "a" (attention) / "l" (late)

    with TileContext(nc) as tc:
        with (
            tc.tile_pool(name="consts", bufs=1) as cpool,
            tc.tile_pool(name="wst", bufs=4) as wstp,
            tc.tile_pool(name="perm", bufs=1) as ppool,
            tc.tile_pool(name="str", bufs=1) as spool,
            tc.tile_pool(name="pp", bufs=2, space="PSUM") as pspool,
            tc.tile_pool(name="pa", bufs=1, space="PSUM") as papool,
            tc.tile_pool(name="pm", bufs=1, space="PSUM") as pmpool,
        ):
            cpk = cpool.tile([128, 336], F32, name="consts")
            cpr = cpool.tile([128, 129], F32R, name="constr")
            ones_row = cpool.tile([1, 128], F32R, name="ones_row")
            nc.sync.dma_start(out=cpk[:], in_=consts_in.ap())
            nc.sync.dma_start(out=cpr[:], in_=constr_in.ap())
            nc.sync.dma_start(out=ones_row[:], in_=ones_row_in.ap())
            ident = cpk[:, 0:128]
            jmat = cpk[:, 128:256]
            rampl = cpk[:, 256:272]
            ones12 = cpk[:, 272:284]
            bias_pc = cpk[:, 284:336]
            identr = cpr[:, 0:128]
            ones_div = cpr[:, 128:129]

            def ps_tile(name):
                return pspool.tile([128, 512], F32, name=name, tag="pp")

            y_t, x_t, xin_t, tacc_t = {}, {}, {}, {}
            for b in range(BPC):
                tacc_t[b] = ppool.tile([128, NCH * CW], BF16, name=f"tacc{b}",
                                       tag=f"tacc{b}", bufs=1)

            def load_xin(b, dram, split=False):
                t = P["a"].tile([128, NCH * L], F32R, name=f"xin{b}", tag="xin", bufs=1)
                if split:
                    for c2 in range(NCH):
                        nc.sync.dma_start(
                            out=t[:].rearrange("p (c l) -> p c l", c=NCH)[:, c2:c2 + 1],
                            in_=dram.ap()[b].rearrange("(c p) l -> p c l", p=128)
                                [:, c2:c2 + 1])
                else:
                    nc.sync.dma_start(out=t[:].rearrange("p (c l) -> p c l", c=NCH),
                                      in_=dram.ap()[b].rearrange("(c p) l -> p c l", p=128))
                xin_t[b] = t
                return t

            # -------------------- building blocks --------------------
            def wm_load(w_dram, m):
                wm = wstp.tile([128, 512], F32R, name=f"wm{m}", tag="wst")
                nc.sync.dma_start(
                    out=wm[:].rearrange("p (c n) -> p c n", c=NCH),
                    in_=w_dram.ap().rearrange("(c p) n -> p c n", p=128)
                        [:, :, 128 * m : 128 * m + 128])
                return wm

            def proj_el(out_t, src_t, w_dram, bcol, resid=None, resid_dram=None,
                        wpre=None):
                """out[e_out,t] = sum_e w[e,e_out] src[e,t] + bias (+resid)."""
                for m in range(NCH):
                    wm = wpre[m] if wpre is not None else wm_load(w_dram, m)
                    for h in range(2):
                        pt = ps_tile(f"pp{m}{h}")
                        first = True
                        if resid is not None:
                            nc.tensor.matmul(pt[:], identr,
                                             resid[:, m * L + 512 * h : m * L + 512 * h + 512],
                                             start=True, stop=False)
                            first = False
                        elif resid_dram is not None:
                            rs = wstp.tile([128, 512], F32R, name=f"rs{m}{h}", tag="wst")
                            nc.scalar.dma_start(
                                out=rs[:],
                                in_=resid_dram.rearrange("(c p) l -> p c l", p=128)
                                    [:, m, 512 * h : 512 * h + 512])
                            nc.tensor.matmul(pt[:], identr, rs[:], start=True, stop=False)
                            first = False
                        for c in range(NCH):
                            nc.tensor.matmul(
                                pt[:], wm[:, c * 128 : c * 128 + 128],
                                src_t[:, c * L + 512 * h : c * L + 512 * h + 512],
                                start=(first and c == 0), stop=(c == NCH - 1))
                        dst = out_t[:, m * L + 512 * h : m * L + 512 * h + 512]
                        if (m + h) % 2 == 0:
                            nc.vector.tensor_scalar(dst, pt[:],
                                bias_pc[:, bcol + m : bcol + m + 1], None, op0=OP.add)
                        else:
                            nc.scalar.activation(dst, pt[:], AF.Identity,
                                bias=bias_pc[:, bcol + m : bcol + m + 1])

            def proj_rev(out_t, src_dram, w_dram):
                """time-reversed V in (L,E): out[s',e] = sum_e' xrev[e',s'] w[e',e].
                xrev streamed from DRAM in 128-wide t' chunks; bv folded into bo."""
                wv = []
                for c in range(NCH):
                    wc = wstp.tile([128, 512], F32R, name=f"wvc{c}", tag="wst")
                    nc.sync.dma_start(
                        out=wc[:],
                        in_=w_dram.ap().rearrange("(c p) n -> p c n", p=128)[:, c, :])
                    wv.append(wc)
                for j in range(NT):
                    rs = P["a"].tile([128, NCH * 128], F32R, name=f"rv{j}", tag="rv",
                                     bufs=3)
                    nc.scalar.dma_start(
                        out=rs[:].rearrange("p (c l) -> p c l", c=NCH),
                        in_=src_dram.rearrange("(c p) l -> p c l", p=128)
                            [:, :, 128 * j : 128 * j + 128])
                    pt = ps_tile(f"pv{j}")
                    for c in range(NCH):
                        nc.tensor.matmul(pt[:], rs[:, c * 128 : c * 128 + 128],
                                         wv[c][:], start=(c == 0), stop=(c == NCH - 1))
                    if j % 2 == 0:
                        nc.vector.tensor_copy(out_t[:, j * E : (j + 1) * E], pt[:])
                    else:
                        nc.scalar.activation(out_t[:, j * E : (j + 1) * E], pt[:], AF.Copy)

            def scores(b, pfx, q_t, k_t):
                """Band-accumulated scores: pa[r, j] = sum_i G[128i+r, j+128i],
                then one DRAM shear round-trip."""
                pa = papool.tile([128, 1152], F32, name=f"pa{pfx}{b}", tag="pa")
                for i in range(NT):
                    segs = _score_segments(i)
                    for c in range(NCH):
                        lhs = k_t[:, c * L + 128 * i : c * L + 128 * i + 128]
                        for (j0, j1) in segs:
                            t0 = (j0 + 128 * i) % L
                            nc.tensor.matmul(
                                pa[:, j0:j1], lhs,
                                q_t[:, c * L + t0 : c * L + t0 + (j1 - j0)],
                                start=(i == 0 and c == 0),
                                stop=(i == NT - 1 and c == NCH - 1))
                bsb = P["a"].tile([128, 1152], F32R, name=f"bsb{pfx}{b}", tag="bsb", bufs=1)
                for h in range(3):
                    sl = slice(384 * h, 384 * h + 384)
                    if h % 2 == 0:
                        nc.scalar.activation(bsb[:, sl], pa[:, sl], AF.Copy)
                    else:
                        nc.vector.tensor_copy(bsb[:, sl], pa[:, sl])
                nc.sync.dma_start(out=a2d[(b, pfx)].ap(), in_=bsb[:])
                sch = P["a"].tile([128, L], F32R, name=f"sch{pfx}{b}", tag="sch", bufs=1)
                nc.sync.dma_start(
                    out=sch[:], in_=bass.AP(a2d[(b, pfx)], 0, [[1153, 128], [1, L]]))
                return sch

            def mps_reduce(b, pfx, sch):
                m_ps = pmpool.tile([1, L], F32, name=f"mps{pfx}{b}", tag="psbig")
                for h in range(2):
                    nc.tensor.matmul(m_ps[0:1, 512 * h : 512 * h + 512], ones_div,
                                     sch[:, 512 * h : 512 * h + 512],
                                     start=True, stop=True)
                return m_ps

            def softmax_c2(b, pfx, m_ps):
                """reverse M; scattered top-6 softmax -> c2R written twice to DRAM;
                compact cc tile loaded back (DVE DMA queue, behind its producers)."""
                m_row = spool.tile([1, L], F32, name=f"mrow{pfx}{b}", tag="mrow", bufs=1)
                nc.vector.tensor_copy(m_row[:], m_ps[:])
                colt_ps = pspool.tile([128, 8], F32, name=f"colt{pfx}{b}", tag="pp")
                for g in range(8):
                    nc.tensor.transpose(colt_ps[:, g : g + 1],
                                        m_row[0:1, 128 * g : 128 * g + 128], ident[0:1, 0:1])
                colt = spool.tile([128, 8], F32, name=f"coltsb{pfx}{b}", tag="coltsb", bufs=1)
                nc.vector.tensor_copy(colt[:], colt_ps[:])
                revt_ps = pspool.tile([128, 8], F32, name=f"revt{pfx}{b}", tag="pp")
                nc.tensor.matmul(revt_ps[:], jmat, colt[:], start=True, stop=True)
                revt = spool.tile([128, 8], F32, name=f"revtsb{pfx}{b}", tag="revtsb", bufs=1)
                nc.vector.tensor_copy(revt[:], revt_ps[:])
                mrev_ps = pmpool.tile([1, L], F32, name=f"mrevps{pfx}{b}", tag="psbig")
                for g in range(8):
                    nc.tensor.transpose(mrev_ps[0:1, 128 * (7 - g) : 128 * (7 - g) + 128],
                                        revt[:, g : g + 1], ident[:, :])
                mrev = spool.tile([1, L], F32, name=f"mrev{pfx}{b}", tag="mrev", bufs=1)
                nc.vector.tensor_copy(mrev[:], mrev_ps[:])

                top8 = spool.tile([1, 8], F32, name=f"top8{pfx}{b}", tag="top8", bufs=1)
                nc.vector.max(top8[:], mrev[:])
                negmax = spool.tile([1, 1], F32, name=f"ngm{pfx}{b}", tag="ngm", bufs=1)
                nc.vector.tensor_scalar(negmax[:], top8[:, 0:1], -1.0, None, op0=OP.mult)
                nc.vector.tensor_scalar(m_row[:], mrev[:], top8[:, 5:6], None, op0=OP.is_ge)
                nc.scalar.activation(mrev[:], mrev[:], AF.Exp, bias=negmax[:, 0:1])
                nc.vector.tensor_tensor(mrev[:], m_row[:], mrev[:], OP.mult)
                csum = spool.tile([1, 1], F32, name=f"csum{pfx}{b}", tag="csum", bufs=1)
                nc.vector.tensor_reduce(csum[:], mrev[:], axis=mybir.AxisListType.X, op=OP.add)
                crecip = spool.tile([1, 1], F32, name=f"crec{pfx}{b}", tag="crec", bufs=1)
                nc.vector.reciprocal(crecip[:], csum[:])
                c2r_sb = spool.tile([1, L], BF16, name=f"c2r{pfx}{b}", tag="c2r", bufs=1)
                nc.vector.tensor_scalar(c2r_sb[:], mrev[:], crecip[:, 0:1], None, op0=OP.mult)
                nc.scalar.dma_start(out=c2rd[(b, pfx)].ap()[0:L], in_=c2r_sb[:])
                nc.scalar.dma_start(out=c2rd[(b, pfx)].ap()[L:], in_=c2r_sb[:])
                cc = P["a"].tile([128, 1920], BF16, name=f"cc{pfx}{b}", tag="cc", bufs=1)
                nc.scalar.dma_start(out=cc[:], in_=bass.AP(c2rd[(b, pfx)], 0,
                                                           [[1, 128], [1, 1920]]))
                return cc

            def agg_block(b, pfx, vrev_t, cc, out_t):
                """agg[e,t] = sum_{s'} Vrev[s',e] c2R[s'+t] via compact cc tile."""
                for m in range(NCH):
                    for h in range(2):
                        pt = ps_tile(f"pag{m}{h}")
                        for j in range(NT):
                            nc.tensor.matmul(
                                pt[:], vrev_t[:, j * E + 128 * m : j * E + 128 * m + 128],
                                cc[:, 128 * j + 512 * h : 128 * j + 512 * h + 512],
                                start=(j == 0), stop=(j == NT - 1))
                        dsta = out_t[:, m * L + 512 * h : m * L + 512 * h + 512]
                        if (m + h) % 2 == 0:
                            nc.vector.tensor_copy(dsta, pt[:])
                        else:
                            nc.scalar.activation(dsta, pt[:], AF.Copy)

            def decomp(b, y_t_, xnext_t, stage):
                """xnext = y - movavg25(y); tacc (+)= movavg25(y) (bf16 SBUF).
                Scan chains (AP-scalar ops) on DVE; the heavy immediate-scalar
                ops (xnext, tacc) on gpsimd, xnext first."""
                tacc = tacc_t[b]
                ws_l = []
                for c in range(NCH):
                    y = lambda a, bb, c=c: y_t_[:, c * L + a : c * L + bb]
                    ws = spool.tile([128, L], F32, name=f"ws{c}", tag="ws", bufs=4)
                    cs25 = spool.tile([128, 25], F32, name=f"cs25{c}", tag=f"cs25{c}", bufs=1)
                    nc.vector.tensor_tensor_scan(cs25[:], y(0, 25), y(0, 25), 0.0,
                                                 op0=OP.add, op1=OP.bypass)
                    nc.vector.scalar_tensor_tensor(ws[:, 0:13], rampl[:, 0:13], y(0, 1),
                                                   cs25[:, 12:25], op0=OP.mult, op1=OP.add)
                    nc.vector.tensor_tensor_scan(ws[:, 13:1012], y(25, L), y(0, 999),
                                                 cs25[:, 24:25], op0=OP.add, op1=OP.subtract)
                    ylast = spool.tile([128, 12], F32, name=f"yl{c}", tag=f"yl{c}", bufs=1)
                    nc.vector.tensor_scalar(ylast[:], ones12, y(L - 1, L), None,
                                            op0=OP.mult)
                    nc.vector.tensor_tensor_scan(ws[:, 1012:1024], ylast[:], y(999, 1011),
                                                 ws[:, 1011:1012], op0=OP.add, op1=OP.subtract)
                    nc.vector.scalar_tensor_tensor(
                        xnext_t[:, c * L : (c + 1) * L], ws[:], -1.0 / KS, y(0, L),
                        op0=OP.mult, op1=OP.add)
                    ws_l.append(ws)
                for c in range(NCH):
                    ws = ws_l[c]
                    tsl = tacc[:, c * CW + 1 : c * CW + 1 + L]
                    if stage == 0:
                        nc.gpsimd.tensor_scalar(tsl, ws[:], 1.0 / KS, None, op0=OP.mult)
                    else:
                        wss = spool.tile([128, L], BF16, name=f"wss{c}", tag="wss", bufs=1)
                        nc.gpsimd.tensor_scalar(wss[:], ws[:], 1.0 / KS, None, op0=OP.mult)
                        nc.gpsimd.tensor_add(tsl, wss[:], tsl)
                    if stage == 2:
                        nc.gpsimd.tensor_copy(tacc[:, c * CW : c * CW + 1],
                                              tacc[:, c * CW + L : c * CW + L + 1])
                        nc.gpsimd.tensor_copy(tacc[:, c * CW + 1025 : c * CW + 1026],
                                              tacc[:, c * CW + 1 : c * CW + 2])

            def ffn_block(b, x3, fcw):
                x3bf = P["l"].tile([128, NCH * L], BF16, name=f"x3bf{b}", tag="x3bf", bufs=1)
                nc.vector.tensor_copy(x3bf[:], x3[:])
                y3 = ppool.tile([128, NCH * L], F32, name=f"y3{b}", tag="y", bufs=2)
                for half in range(2):
                    h_t = P["l"].tile([128, NXP * 512], BF16, name=f"h{b}{half}",
                                      tag="lbuf16", bufs=2)
                    for xc in range(NXP):
                        pt = ps_tile(f"ph{xc}")
                        for c in range(NCH):
                            nc.tensor.matmul(
                                pt[:], fcw[:, xc * 512 + 128 * c : xc * 512 + 128 * c + 128],
                                x3bf[:, c * L + 512 * half : c * L + 512 * half + 512],
                                start=(c == 0), stop=(c == NCH - 1))
                        nc.scalar.activation(h_t[:, xc * 512 : (xc + 1) * 512], pt[:],
                                             AF.Gelu, bias=bias_pc[:, FC1B + xc : FC1B + xc + 1])
                    for m in range(NCH):
                        pt = ps_tile(f"pf{m}")
                        for xc in range(NXP):
                            nc.tensor.matmul(
                                pt[:],
                                fcw[:, 8192 + m * 2048 + 128 * xc : 8192 + m * 2048 + 128 * xc + 128],
                                h_t[:, xc * 512 : (xc + 1) * 512],
                                start=(xc == 0), stop=(xc == NXP - 1))
                        sl = slice(m * L + 512 * half, m * L + 512 * half + 512)
                        nc.vector.scalar_tensor_tensor(
                            y3[:, sl], pt[:], bias_pc[:, FC2B + m : FC2B + m + 1],
                            x3[:, sl], op0=OP.add, op1=OP.add)
                return y3

            def ln_stats(b, x4):
                sq = P["l"].tile([128, NCH * L], F32R, name=f"sq{b}", tag="lbuf16", bufs=2)
                for c in range(NCH):
                    nc.scalar.activation(sq[:, c * L : (c + 1) * L],
                                         x4[:, c * L : (c + 1) * L], AF.Square)
                mu_ps = pmpool.tile([1, L], F32, name=f"mups{b}", tag="psbig")
                for h in range(2):
                    for c in range(NCH):
                        nc.tensor.matmul(mu_ps[0:1, 512 * h : 512 * h + 512], ones_div,
                                         x4[:, c * L + 512 * h : c * L + 512 * h + 512],
                                         start=(c == 0), stop=(c == NCH - 1))
                mu_r = spool.tile([1, L], F32, name=f"mur{b}", tag="mrow", bufs=1)
                nc.vector.tensor_copy(mu_r[:], mu_ps[:])
                ms_ps = pmpool.tile([1, L], F32, name=f"msps{b}", tag="psbig")
                for h in range(2):
                    for c in range(NCH):
                        nc.tensor.matmul(ms_ps[0:1, 512 * h : 512 * h + 512], ones_div,
                                         sq[:, c * L + 512 * h : c * L + 512 * h + 512],
                                         start=(c == 0), stop=(c == NCH - 1))
                var_r = spool.tile([1, L], F32, name=f"varr{b}", tag="mrev", bufs=1)
                nc.vector.tensor_tensor(var_r[:], mu_r[:], mu_r[:], OP.mult)
                nc.vector.scalar_tensor_tensor(var_r[:], ms_ps[:], 1e-5, var_r[:],
                                               op0=OP.add, op1=OP.subtract)
                nc.scalar.activation(var_r[:], var_r[:], AF.Sqrt)
                rows = spool.tile([1, L], F32R, name=f"rows{b}", tag="c2r", bufs=1)
                rows2 = spool.tile([1, L], F32R, name=f"rows2{b}", tag="rows2", bufs=1)
                with nc.allow_low_precision(reason="istd broadcast is f32r by design"):
                    nc.vector.reciprocal(rows[:], var_r[:])
                nc.vector.tensor_tensor(rows2[:], mu_r[:], rows[:], OP.mult)
                return rows, rows2

            def ln_apply(b, x4, rows, rows2):
                bc = P["l"].tile([128, 2 * L], F32, name=f"bc{b}", tag="lbuf16", bufs=2)
                for h in range(4):
                    bp = ps_tile(f"bc{h}")
                    src_row = rows if h < 2 else rows2
                    nc.tensor.matmul(bp[:], ones_row[:],
                                     src_row[0:1, 512 * (h % 2) : 512 * (h % 2) + 512],
                                     start=True, stop=True)
                    if h % 2 == 0:
                        nc.vector.tensor_copy(bc[:, 512 * h : 512 * h + 512], bp[:])
                    else:
                        nc.scalar.activation(bc[:, 512 * h : 512 * h + 512], bp[:], AF.Copy)
                seas = ppool.tile([128, NCH * L], F32, name=f"seas{b}", tag="y", bufs=2)
                accs = spool.tile([128, NCH], F32, name=f"accs{b}", tag="accs", bufs=2)
                for c in range(NCH):
                    t1 = spool.tile([128, L], F32, name=f"t1{c}", tag="ws", bufs=4)
                    nc.vector.tensor_tensor(t1[:], x4[:, c * L : (c + 1) * L],
                                            bc[:, 0:L], OP.mult)
                    nc.vector.tensor_tensor(t1[:], t1[:], bc[:, L:], OP.subtract)
                    nc.scalar.activation(seas[:, c * L : (c + 1) * L], t1[:], AF.Identity,
                                         bias=bias_pc[:, LNB + c : LNB + c + 1],
                                         scale=bias_pc[:, LNG + c : LNG + c + 1],
                                         accum_out=accs[:, c : c + 1])
                nc.vector.tensor_scalar(accs[:], accs[:], -1.0 / L, None, op0=OP.mult)
                for c in range(NCH):
                    if c % 2 == 0:
                        nc.scalar.activation(seas[:, c * L : (c + 1) * L],
                                             seas[:, c * L : (c + 1) * L], AF.Identity,
                                             bias=accs[:, c : c + 1])
                    else:
                        nc.vector.tensor_scalar(seas[:, c * L : (c + 1) * L],
                                                seas[:, c * L : (c + 1) * L],
                                                accs[:, c : c + 1], None, op0=OP.add)
                return seas

            def seas_out_block(b, seas):
                for a in range(NT):
                    tp = ps_tile(f"tps{a}")
                    for c in range(NCH):
                        nc.tensor.transpose(tp[:, 128 * c : 128 * c + 128],
                                            seas[:, c * L + 128 * a : c * L + 128 * a + 128],
                                            ident[:, :])
                    osb = spool.tile([128, 512], F32, name=f"osb{a}", tag="osb", bufs=2)
                    nc.scalar.activation(osb[:], tp[:], AF.Copy)
                    nc.sync.dma_start(out=seas_out.ap()[b, 128 * a : 128 * a + 128, :],
                                      in_=osb[:])

            def trend_conv(b, wct, rng=None):
                tacc = tacc_t[b]
                for a in (rng if rng is not None else range(NT)):
                    pt = ps_tile(f"ptc{a}")
                    n = 0
                    for j in range(3):
                        for c in range(NCH):
                            nc.tensor.matmul(
                                pt[:],
                                tacc[:, c * CW + 128 * a + j : c * CW + 128 * a + j + 128],
                                wct[:, (j * NCH + c) * F : (j * NCH + c) * F + F],
                                start=(n == 0), stop=(n == 11))
                            n += 1
                    osb = spool.tile([128, 512], F32, name=f"osc{a}", tag="osb", bufs=2)
                    nc.scalar.activation(osb[:], pt[:], AF.Copy)
                    nc.sync.dma_start(out=trend_out.ap()[b, 128 * a : 128 * a + 128, :],
                                      in_=osb[:])

            # ============== attention phase: hand-scheduled emission ==============
            q_t, k_t, v_t, cc_t, sch_t = {}, {}, {}, {}, {}

            def attn_head(b, pfx, wpre=None):
                """proj q,k + band scores + shear round trip."""
                _mark(nc, f"{pfx}{b}.proj_qk")
                xq = x_t[b] if pfx == "ca" else xin_t[b]
                q = P["a"].tile([128, NCH * L], F32R, name=f"q{pfx}{b}", tag="q", bufs=1)
                k = P["a"].tile([128, NCH * L], F32R, name=f"k{pfx}{b}", tag="k", bufs=1)
                proj_el(q, xq, w_in[f"{pfx}_wq"], BQ[pfx], wpre=wpre)
                proj_el(k, xin_t[b], w_in[f"{pfx}_wk"], BK[pfx])
                _mark(nc, f"{pfx}{b}.scores")
                sch_t[b] = scores(b, pfx, q, k)

            def attn_mps(b, pfx):
                _mark(nc, f"{pfx}{b}.mps")
                mp = mps_reduce(b, pfx, sch_t[b])
                _mark(nc, f"{pfx}{b}.softmax")
                cc_t[b] = softmax_c2(b, pfx, mp)

            def attn_projv(b, pfx):
                _mark(nc, f"{pfx}{b}.projv")
                v = P["a"].tile([128, NT * E], BF16, name=f"v{pfx}{b}", tag="v", bufs=1)
                proj_rev(v, (xtr_in if pfx == "sa" else entr_in).ap()[b],
                         w_in[f"{pfx}_wv"])
                v_t[b] = v

            ag_t = {}

            def attn_agg(b, pfx):
                _mark(nc, f"{pfx}{b}.agg")
                ag = P["a"].tile([128, NCH * L], F32R, name=f"agg{pfx}{b}", tag="q", bufs=1)
                agg_block(b, pfx, v_t[b], cc_t[b], ag)
                ag_t[b] = ag

            def attn_projo(b, pfx, stage, resid=None, resid_dram=None):
                _mark(nc, f"{pfx}{b}.projo")
                y = ppool.tile([128, NCH * L], F32, name=f"y{pfx}{b}", tag="y", bufs=2)
                proj_el(y, ag_t[b], w_in[f"{pfx}_wo"], BO[pfx], resid=resid,
                        resid_dram=resid_dram)
                _mark(nc, f"decomp{b}")
                xn = ppool.tile([128, NCH * L], F32R, name=f"x{pfx}{b}", tag="x", bufs=2)
                decomp(b, y, xn, stage)
                x_t[b] = xn

            def attn_tail(b, pfx, stage, resid=None, resid_dram=None):
                attn_agg(b, pfx)
                attn_projo(b, pfx, stage, resid=resid, resid_dram=resid_dram)

            with tc.tile_pool(name="attn", bufs=1) as atpool:
                P["a"] = atpool
                _mark(nc, "init")
                wpre0 = [wm_load(w_in["sa_wq"], m) for m in range(NCH)]
                load_xin(0, xt_in, split=True)
                attn_head(0, "sa", wpre0)
                load_xin(1, xt_in)
                attn_projv(0, "sa")
                attn_mps(0, "sa")
                attn_head(1, "sa")
                attn_tail(0, "sa", 0, resid_dram=xt_in.ap()[0])
                attn_projv(1, "sa")
                load_xin(0, ent_in)
                attn_mps(1, "sa")
                attn_head(0, "ca")                   # needs x_t[0] from decomp0
                attn_tail(1, "sa", 0, resid_dram=xt_in.ap()[1])
                attn_projv(0, "ca")
                load_xin(1, ent_in)
                attn_mps(0, "ca")
                attn_head(1, "ca")
                attn_agg(0, "ca")
                attn_mps(1, "ca")
                attn_projo(0, "ca", 1, resid=x_t[0])
                attn_projv(1, "ca")
                attn_tail(1, "ca", 1, resid=x_t[1])

            with tc.tile_pool(name="late", bufs=1) as ltpool:
                P["l"] = ltpool
                _mark(nc, "ffn_w")
                fcw = ltpool.tile([128, 16384], BF16, name="fcw", tag="bigw", bufs=1)
                for j in range(8):
                    nc.sync.dma_start(out=fcw[:, j * 2048 : (j + 1) * 2048],
                                      in_=fcw_in.ap()[:, j * 2048 : (j + 1) * 2048])
                _mark(nc, "ffn0")
                y0 = ffn_block(0, x_t[0], fcw)
                _mark(nc, "decomp0")
                x40 = ppool.tile([128, NCH * L], F32R, name="x40", tag="x", bufs=2)
                decomp(0, y0, x40, 2)
                x_t[0] = x40
                _mark(nc, "ffn1")
                y1 = ffn_block(1, x_t[1], fcw)
                _mark(nc, "ln0a")
                r0, r20 = ln_stats(0, x40)
                _mark(nc, "decomp1")
                x41 = ppool.tile([128, NCH * L], F32R, name="x41", tag="x", bufs=2)
                decomp(1, y1, x41, 2)
                x_t[1] = x41
                _mark(nc, "trend_w")
                wct = ltpool.tile([128, 12 * F], BF16, name="wctt", tag="bigw", bufs=1)
                nc.sync.dma_start(out=wct[:], in_=wct_in.ap())
                _mark(nc, "trend0")
                trend_conv(0, wct)
                _mark(nc, "ln1a")
                r1, r21 = ln_stats(1, x41)
                _mark(nc, "ln0b")
                seas0 = ln_apply(0, x40, r0, r20)
                _mark(nc, "trend1")
                trend_conv(1, wct, rng=range(0, 4))
                _mark(nc, "ln0out")
                seas_out_block(0, seas0)
                _mark(nc, "ln1b")
                seas1 = ln_apply(1, x41, r1, r21)
                _mark(nc, "trend1b")
                trend_conv(1, wct, rng=range(4, NT))
                _mark(nc, "ln1out")
                seas_out_block(1, seas1)

    nc.compile()
    return nc


def _host_prep(inputs):
    f32 = np.float32
    import ml_dtypes
    x = np.asarray(inputs["x"], f32)
    enc = np.asarray(inputs["enc_output"], f32)
    xt = np.ascontiguousarray(x.transpose(0, 2, 1))
    xtr = np.ascontiguousarray(xt[:, :, ::-1])
    ent = np.ascontiguousarray(enc.transpose(0, 2, 1))
    entr = np.ascontiguousarray(ent[:, :, ::-1])

    shared = {}
    for p in ("sa", "ca"):
        for nme in ("wq", "wk", "wv", "wo"):
            shared[f"{p}_{nme}"] = np.ascontiguousarray(np.asarray(inputs[f"{p}_{nme}"], f32))
    fc1 = np.asarray(inputs["fc1_w"], f32).reshape(NCH, 128, XP)       # (c, p, xp)
    fc2 = np.asarray(inputs["fc2_w"], f32).reshape(NXP, 128, E)        # (xc, p, e)
    fcw = np.zeros((128, 16384), ml_dtypes.bfloat16)
    fcw[:, :8192] = fc1.reshape(NCH, 128, NXP, 128).transpose(1, 2, 0, 3) \
        .reshape(128, 8192).astype(ml_dtypes.bfloat16)
    fcw[:, 8192:] = fc2.reshape(NXP, 128, NCH, 128).transpose(1, 2, 0, 3) \
        .reshape(128, 8192).astype(ml_dtypes.bfloat16)
    shared["fcw"] = fcw
    tw = np.asarray(inputs["trend_w"], f32)
    # [p, (j c), f]: row (j*4+c)*128+p of the (3E, F) matrix = w[(j e)], e=128c+p
    wct = tw.transpose(2, 1, 0).reshape(3 * E, F)                      # [(j e), f]
    shared["wct"] = np.ascontiguousarray(
        wct.reshape(12, 128, F).transpose(1, 0, 2).reshape(128, 12 * F)
    ).astype(ml_dtypes.bfloat16)

    def pc(v, nch=4):
        return np.ascontiguousarray(np.asarray(v, f32).reshape(nch, 128).T)

    # fold bv into bo: out = Wo^T (agg + bv) + bo = Wo^T agg + (bv @ Wo + bo)
    sa_bo2 = np.asarray(inputs["sa_bv"], f32) @ np.asarray(inputs["sa_wo"], f32) \
        + np.asarray(inputs["sa_bo"], f32)
    ca_bo2 = np.asarray(inputs["ca_bv"], f32) @ np.asarray(inputs["ca_wo"], f32) \
        + np.asarray(inputs["ca_bo"], f32)
    shared["bias_pc"] = np.ascontiguousarray(np.concatenate([
        pc(inputs["sa_bq"]), pc(inputs["sa_bk"]), pc(sa_bo2),
        pc(inputs["ca_bq"]), pc(inputs["ca_bk"]), pc(ca_bo2),
        pc(inputs["fc2_b"]), pc(inputs["ln_g"]), pc(inputs["ln_b"]),
        pc(inputs["fc1_b"], 16),
    ], axis=1))
    ramp = np.zeros(16, f32)
    ramp[:13] = np.arange(12, -1, -1)
    shared["consts"] = np.ascontiguousarray(np.concatenate([
        np.eye(128, dtype=f32),                       # ident 0:128
        np.eye(128, dtype=f32)[::-1],                 # jmat 128:256
        np.tile(ramp, (128, 1)),                      # rampl 256:272
        np.ones((128, 12), f32),                      # ones12 272:284
        shared.pop("bias_pc"),                        # bias_pc 284:336
    ], axis=1))
    shared["constr"] = np.ascontiguousarray(np.concatenate([
        np.eye(128, dtype=f32),                       # identr 0:128
        np.full((128, 1), 1.0 / E, f32),              # ones_div 128:129
    ], axis=1))
    shared["ones_row"] = np.ones((1, 128), f32)

    in_maps = []
    for core in range(NCORES):
        s = slice(core * BPC, (core + 1) * BPC)
        m = dict(shared)
        m["xt"] = np.ascontiguousarray(xt[s])
        m["xtr"] = np.ascontiguousarray(xtr[s])
        m["ent"] = np.ascontiguousarray(ent[s])
        m["entr"] = np.ascontiguousarray(entr[s])
        in_maps.append(m)
    return in_maps


_LAST = {}


def kernel(**inputs):
    from concourse.bass_utils import run_bass_kernel_spmd

    nc = _build()
    in_maps = _host_prep(inputs)
    res = run_bass_kernel_spmd(nc, in_maps, core_ids=list(range(NCORES)),
                               **_LAST.get("kwargs", {}))
    _LAST["res"] = res
    seasonal = np.concatenate([res.results[c]["seasonal"] for c in range(NCORES)], axis=0)
    trend = np.concatenate([res.results[c]["trend"] for c in range(NCORES)], axis=0)
    return seasonal, trend


# revision 19
# speedup vs baseline: 1.6159x; 1.0139x over previous
"""Trainium2 Bass kernel for nn_DecoderBlock (Autoformer decoder layer).

Data-parallel over batch: 16 batches -> 8 cores x 2 batches. Layout is
channel-major (E on partitions). The two per-core batches are software-
pipelined: emission order is hand-scheduled so the in-order PE queue never
waits on the softmax/DMA latency chains of the other batch.

Autocorrelation without FFTs:
  M[tau] = (1/E) sum_t <k_t, q_{(t+tau)%L}>
Key identity: the Gram element (s, s+tau) with s = 128*i + r lands at band
column j = r + tau for EVERY i-block, so all 8 i-blocks accumulate into ONE
PSUM band tile [128, 1152]; a single DRAM shear round-trip (row stride 1153
on readback) + one ones-matmul yields M.  Top-6 + softmax give a scattered
weight vector; the roll-aggregation
  agg[e,t] = sum_{s'} Vrev[s',e] * c2R[s'+t]
uses host-time-reversed V inputs and the reversed weight vector written
twice to DRAM, read back as a compact [128, 1920] sliding-window tile.

Trend: t1+t2+t3 accumulates in an SBUF bf16 tile (per-batch) written by the
decomposition stages; the circular conv reads it directly.

Precision: score matmuls in f32r (top-6 margins ~1e-4 in M units); FFN and
trend conv in bf16; everything else f32r.
"""
import functools
import numpy as np

NCORES = 8
BPC = 2
L = 1024
E = 512
XP = 2048
F = 512
KS = 25
NCH = 4
NT = 8
NXP = 16

PHASES = []


def _mark(nc, label):
    PHASES.append((label, nc.next_id()))


def _score_segments(i):
    """Column segments of the band [0, 1152) for k-block i, cut at the q wrap
    point jw = L - 128*i, each segment <=512 wide and >=256 where possible."""
    jw = L - 128 * i
    segs = []
    for lo, hi in ((0, jw), (jw, 1152)):
        w = hi - lo
        if w <= 0:
            continue
        n = (w + 511) // 512
        base = w // n
        rem = w - base * n
        st = lo
        for p in range(n):
            ln = base + (1 if p < rem else 0)
            segs.append((st, st + ln))
            st += ln
    return segs


@functools.lru_cache(maxsize=1)
def _build():
    import concourse.bacc as bacc
    import concourse.bass as bass
    import concourse.mybir as mybir
    from concourse.tile import TileContext

    F32 = mybir.dt.float32
    F32R = mybir.dt.float32r
    BF16 = mybir.dt.bfloat16
    AF = mybir.ActivationFunctionType
    OP = mybir.AluOpType

    nc = bacc.Bacc("TRN2", target_bir_lowering=False, debug=False, num_devices=NCORES)

    def din(name, shape, dtype=F32R):
        return nc.declare_dram_parameter(name, list(shape), dtype, isOutput=False)

    xt_in = din("xt", (BPC, E, L))
    xtr_in = din("xtr", (BPC, E, L))
    ent_in = din("ent", (BPC, E, L))
    entr_in = din("entr", (BPC, E, L))
    w_in = {}
    for p in ("sa", "ca"):
        for nme in ("wq", "wk", "wv", "wo"):
            w_in[f"{p}_{nme}"] = din(f"{p}_{nme}", (E, E))
    fcw_in = din("fcw", (128, 16384), BF16)   # packed fc1(c,2048)|fc2(xc,512), bf16
    wct_in = din("wct", (128, 12 * F), BF16)  # conv weights, bf16, [p, (j c), f]
    consts_in = din("consts", (128, 336), F32)
    constr_in = din("constr", (128, 129), F32R)
    ones_row_in = din("ones_row", (1, 128))

    seas_out = nc.declare_dram_parameter("seasonal", [BPC, L, E], F32, isOutput=True)
    trend_out = nc.declare_dram_parameter("trend", [BPC, L, F], F32, isOutput=True)

    a2d = {(b, p): nc.dram_tensor(f"a2d_{p}{b}", [128, 1152], F32R)
           for b in range(BPC) for p in ("sa", "ca")}
    c2rd = {(b, p): nc.dram_tensor(f"c2rd_{p}{b}", [2 * L], BF16)
            for b in range(BPC) for p in ("sa", "ca")}

    BQ = {"sa": 0, "ca": 12}
    BK = {"sa": 4, "ca": 16}
    BO = {"sa": 8, "ca": 20}
    FC2B, LNG, LNB, FC1B = 24, 28, 32, 36
    CW = 1026  # per-channel tacc row: [wrap | 1024 | wrap]

    P = {}  # current phase-scoped pool under key "a" (attention) / "l" (late)

    with TileContext(nc) as tc:
        with (
            tc.tile_pool(name="consts", bufs=1) as cpool,
            tc.tile_pool(name="wst", bufs=4) as wstp,
            tc.tile_pool(name="perm", bufs=1) as ppool,
            tc.tile_pool(name="str", bufs=1) as spool,
            tc.tile_pool(name="pp", bufs=2, space="PSUM") as pspool,
            tc.tile_pool(name="pa", bufs=1, space="PSUM") as papool,
            tc.tile_pool(name="pm", bufs=1, space="PSUM") as pmpool,
        ):
            cpk = cpool.tile([128, 336], F32, name="consts")
            cpr = cpool.tile([128, 129], F32R, name="constr")
            ones_row = cpool.tile([1, 128], F32R, name="ones_row")
            nc.sync.dma_start(out=cpk[:], in_=consts_in.ap())
            nc.sync.dma_start(out=cpr[:], in_=constr_in.ap())
            nc.sync.dma_start(out=ones_row[:], in_=ones_row_in.ap())
            ident = cpk[:, 0:128]
            jmat = cpk[:, 128:256]
            rampl = cpk[:, 256:272]
            ones12 = cpk[:, 272:284]
            bias_pc = cpk[:, 284:336]
            identr = cpr[:, 0:128]
            ones_div = cpr[:, 128:129]

            def ps_tile(name):
                return pspool.tile([128, 512], F32, name=name, tag="pp")

            y_t, x_t, xin_t, tacc_t = {}, {}, {}, {}
            for b in range(BPC):
                tacc_t[b] = ppool.tile([128, NCH * CW], BF16, name=f"tacc{b}",
                                       tag=f"tacc{b}", bufs=1)

            def load_xin(b, dram, split=False):
                ts = []
                for c2 in range(NCH):
                    t = P["a"].tile([128, L], F32R, name=f"xin{b}c{c2}", tag="xin",
                                    bufs=4)
                    nc.sync.dma_start(
                        out=t[:],
                        in_=dram.ap()[b].rearrange("(c p) l -> p c l", p=128)
                            [:, c2])
                    ts.append(t)
                xin_t[b] = ts
                return ts

            # -------------------- building blocks --------------------
            def wm_load(w_dram, m):
                wm = wstp.tile([128, 512], F32R, name=f"wm{m}", tag="wst")
                nc.sync.dma_start(
                    out=wm[:].rearrange("p (c n) -> p c n", c=NCH),
                    in_=w_dram.ap().rearrange("(c p) n -> p c n", p=128)
                        [:, :, 128 * m : 128 * m + 128])
                return wm

            def s_ap(src, c, a, b):
                if isinstance(src, list):
                    return src[c][:, a:b]
                return src[:, c * L + a : c * L + b]

            def proj_el(out_t, src_t, w_dram, bcol, resid=None, resid_dram=None,
                        wpre=None):
                """out[e_out,t] = sum_e w[e,e_out] src[e,t] + bias (+resid)."""
                for m in range(NCH):
                    wm = wpre[m] if wpre is not None else wm_load(w_dram, m)
                    for h in range(2):
                        pt = ps_tile(f"pp{m}{h}")
                        first = True
                        if resid is not None:
                            nc.tensor.matmul(pt[:], identr,
                                             resid[:, m * L + 512 * h : m * L + 512 * h + 512],
                                             start=True, stop=False)
                            first = False
                        elif resid_dram is not None:
                            rs = wstp.tile([128, 512], F32R, name=f"rs{m}{h}", tag="wst")
                            nc.sync.dma_start(
                                out=rs[:],
                                in_=resid_dram.rearrange("(c p) l -> p c l", p=128)
                                    [:, m, 512 * h : 512 * h + 512])
                            nc.tensor.matmul(pt[:], identr, rs[:], start=True, stop=False)
                            first = False
                        for c in range(NCH):
                            nc.tensor.matmul(
                                pt[:], wm[:, c * 128 : c * 128 + 128],
                                s_ap(src_t, c, 512 * h, 512 * h + 512),
                                start=(first and c == 0), stop=(c == NCH - 1))
                        dst = out_t[:, m * L + 512 * h : m * L + 512 * h + 512]
                        if (m + h) % 2 == 0:
                            nc.vector.tensor_scalar(dst, pt[:],
                                bias_pc[:, bcol + m : bcol + m + 1], None, op0=OP.add)
                        else:
                            nc.scalar.activation(dst, pt[:], AF.Identity,
                                bias=bias_pc[:, bcol + m : bcol + m + 1])

            def proj_rev(out_t, src_dram, w_dram):
                """time-reversed V in (L,E): out[s',e] = sum_e' xrev[e',s'] w[e',e].
                xrev streamed from DRAM in 128-wide t' chunks; bv folded into bo."""
                wv = []
                for c in range(NCH):
                    wc = wstp.tile([128, 512], F32R, name=f"wvc{c}", tag="wst")
                    nc.sync.dma_start(
                        out=wc[:],
                        in_=w_dram.ap().rearrange("(c p) n -> p c n", p=128)[:, c, :])
                    wv.append(wc)
                for j2 in range(4):
                    rs = P["a"].tile([128, NCH * 256], F32R, name=f"rv{j2}", tag="rv",
                                     bufs=2)
                    nc.scalar.dma_start(
                        out=rs[:].rearrange("p (c l) -> p c l", c=NCH),
                        in_=src_dram.rearrange("(c p) l -> p c l", p=128)
                            [:, :, 256 * j2 : 256 * j2 + 256])
                    for jj in range(2):
                        j = 2 * j2 + jj
                        pt = ps_tile(f"pv{j}")
                        for c in range(NCH):
                            nc.tensor.matmul(
                                pt[:], rs[:, c * 256 + 128 * jj : c * 256 + 128 * jj + 128],
                                wv[c][:], start=(c == 0), stop=(c == NCH - 1))
                        if j % 2 == 0:
                            nc.vector.tensor_copy(out_t[:, j * E : (j + 1) * E], pt[:])
                        else:
                            nc.scalar.activation(out_t[:, j * E : (j + 1) * E], pt[:],
                                                 AF.Copy)

            def scores(b, pfx, q_t, k_t):
                """Band-accumulated scores: pa[r, j] = sum_i G[128i+r, j+128i],
                then one DRAM shear round-trip."""
                pa = papool.tile([128, 1152], F32, name=f"pa{pfx}{b}", tag="pa")
                for i in range(NT):
                    segs = _score_segments(i)
                    for c in range(NCH):
                        lhs = k_t[:, c * L + 128 * i : c * L + 128 * i + 128]
                        for (j0, j1) in segs:
                            t0 = (j0 + 128 * i) % L
                            nc.tensor.matmul(
                                pa[:, j0:j1], lhs,
                                q_t[:, c * L + t0 : c * L + t0 + (j1 - j0)],
                                start=(i == 0 and c == 0),
                                stop=(i == NT - 1 and c == NCH - 1))
                bsb = P["a"].tile([128, 1152], F32R, name=f"bsb{pfx}{b}", tag="bsb", bufs=1)
                for h in range(3):
                    sl = slice(384 * h, 384 * h + 384)
                    if h % 2 == 0:
                        nc.scalar.activation(bsb[:, sl], pa[:, sl], AF.Copy)
                    else:
                        nc.vector.tensor_copy(bsb[:, sl], pa[:, sl])
                nc.sync.dma_start(out=a2d[(b, pfx)].ap(), in_=bsb[:])
                sch = P["a"].tile([128, L], F32R, name=f"sch{pfx}{b}", tag="sch", bufs=1)
                nc.sync.dma_start(
                    out=sch[:], in_=bass.AP(a2d[(b, pfx)], 0, [[1153, 128], [1, L]]))
                return sch

            def mps_reduce(b, pfx, sch):
                m_ps = pmpool.tile([1, L], F32, name=f"mps{pfx}{b}", tag="psbig")
                for h in range(2):
                    nc.tensor.matmul(m_ps[0:1, 512 * h : 512 * h + 512], ones_div,
                                     sch[:, 512 * h : 512 * h + 512],
                                     start=True, stop=True)
                return m_ps

            def softmax_c2(b, pfx, m_ps):
                """reverse M; scattered top-6 softmax -> c2R written twice to DRAM;
                compact cc tile loaded back (DVE DMA queue, behind its producers)."""
                m_row = spool.tile([1, L], F32, name=f"mrow{pfx}{b}", tag="mrow", bufs=1)
                nc.vector.tensor_copy(m_row[:], m_ps[:])
                colt_ps = pspool.tile([128, 8], F32, name=f"colt{pfx}{b}", tag="pp")
                for g in range(8):
                    nc.tensor.transpose(colt_ps[:, g : g + 1],
                                        m_row[0:1, 128 * g : 128 * g + 128], ident[0:1, 0:1])
                colt = spool.tile([128, 8], F32, name=f"coltsb{pfx}{b}", tag="coltsb", bufs=1)
                nc.vector.tensor_copy(colt[:], colt_ps[:])
                revt_ps = pspool.tile([128, 8], F32, name=f"revt{pfx}{b}", tag="pp")
                nc.tensor.matmul(revt_ps[:], jmat, colt[:], start=True, stop=True)
                revt = spool.tile([128, 8], F32, name=f"revtsb{pfx}{b}", tag="revtsb", bufs=1)
                nc.vector.tensor_copy(revt[:], revt_ps[:])
                mrev_ps = pmpool.tile([1, L], F32, name=f"mrevps{pfx}{b}", tag="psbig")
                for g in range(8):
                    nc.tensor.transpose(mrev_ps[0:1, 128 * (7 - g) : 128 * (7 - g) + 128],
                                        revt[:, g : g + 1], ident[:, :])
                mrev = spool.tile([1, L], F32, name=f"mrev{pfx}{b}", tag="mrev", bufs=1)
                nc.vector.tensor_copy(mrev[:], mrev_ps[:])

                top8 = spool.tile([1, 8], F32, name=f"top8{pfx}{b}", tag="top8", bufs=1)
                nc.vector.max(top8[:], mrev[:])
                negmax = spool.tile([1, 1], F32, name=f"ngm{pfx}{b}", tag="ngm", bufs=1)
                nc.vector.tensor_scalar(negmax[:], top8[:, 0:1], -1.0, None, op0=OP.mult)
                nc.vector.tensor_scalar(m_row[:], mrev[:], top8[:, 5:6], None, op0=OP.is_ge)
                nc.scalar.activation(mrev[:], mrev[:], AF.Exp, bias=negmax[:, 0:1])
                nc.vector.tensor_tensor(mrev[:], m_row[:], mrev[:], OP.mult)
                csum = spool.tile([1, 1], F32, name=f"csum{pfx}{b}", tag="csum", bufs=1)
                nc.vector.tensor_reduce(csum[:], mrev[:], axis=mybir.AxisListType.X, op=OP.add)
                crecip = spool.tile([1, 1], F32, name=f"crec{pfx}{b}", tag="crec", bufs=1)
                nc.vector.reciprocal(crecip[:], csum[:])
                c2r_sb = spool.tile([1, L], BF16, name=f"c2r{pfx}{b}", tag="c2r", bufs=1)
                nc.vector.tensor_scalar(c2r_sb[:], mrev[:], crecip[:, 0:1], None, op0=OP.mult)
                nc.scalar.dma_start(out=c2rd[(b, pfx)].ap()[0:L], in_=c2r_sb[:])
                nc.scalar.dma_start(out=c2rd[(b, pfx)].ap()[L:], in_=c2r_sb[:])
                cc = P["a"].tile([128, 1920], BF16, name=f"cc{pfx}{b}", tag="cc", bufs=1)
                nc.scalar.dma_start(out=cc[:], in_=bass.AP(c2rd[(b, pfx)], 0,
                                                           [[1, 128], [1, 1920]]))
                return cc

            def agg_block(b, pfx, vrev_t, cc, out_t):
                """agg[e,t] = sum_{s'} Vrev[s',e] c2R[s'+t] via compact cc tile."""
                for m in range(NCH):
                    for h in range(2):
                        pt = ps_tile(f"pag{m}{h}")
                        for j in range(NT):
                            nc.tensor.matmul(
                                pt[:], vrev_t[:, j * E + 128 * m : j * E + 128 * m + 128],
                                cc[:, 128 * j + 512 * h : 128 * j + 512 * h + 512],
                                start=(j == 0), stop=(j == NT - 1))
                        dsta = out_t[:, m * L + 512 * h : m * L + 512 * h + 512]
                        if (m + h) % 2 == 0:
                            nc.vector.tensor_copy(dsta, pt[:])
                        else:
                            nc.scalar.activation(dsta, pt[:], AF.Copy)

            def decomp(b, y_t_, xnext_t, stage):
                """xnext = y - movavg25(y); tacc (+)= movavg25(y) (bf16 SBUF).
                Scan chains (AP-scalar ops) on DVE; the heavy immediate-scalar
                ops (xnext, tacc) on gpsimd, xnext first."""
                tacc = tacc_t[b]
                ws_l = []
                for c in range(NCH):
                    y = lambda a, bb, c=c: y_t_[:, c * L + a : c * L + bb]
                    ws = spool.tile([128, L], F32, name=f"ws{c}", tag="ws", bufs=4)
                    cs25 = spool.tile([128, 25], F32, name=f"cs25{c}", tag=f"cs25{c}", bufs=1)
                    nc.vector.tensor_tensor_scan(cs25[:], y(0, 25), y(0, 25), 0.0,
                                                 op0=OP.add, op1=OP.bypass)
                    nc.vector.scalar_tensor_tensor(ws[:, 0:13], rampl[:, 0:13], y(0, 1),
                                                   cs25[:, 12:25], op0=OP.mult, op1=OP.add)
                    nc.vector.tensor_tensor_scan(ws[:, 13:1012], y(25, L), y(0, 999),
                                                 cs25[:, 24:25], op0=OP.add, op1=OP.subtract)
                    ylast = spool.tile([128, 12], F32, name=f"yl{c}", tag=f"yl{c}", bufs=1)
                    nc.vector.tensor_scalar(ylast[:], ones12, y(L - 1, L), None,
                                            op0=OP.mult)
                    nc.vector.tensor_tensor_scan(ws[:, 1012:1024], ylast[:], y(999, 1011),
                                                 ws[:, 1011:1012], op0=OP.add, op1=OP.subtract)
                    nc.vector.scalar_tensor_tensor(
                        xnext_t[:, c * L : (c + 1) * L], ws[:], -1.0 / KS, y(0, L),
                        op0=OP.mult, op1=OP.add)
                    ws_l.append(ws)
                for c in range(NCH):
                    ws = ws_l[c]
                    tsl = tacc[:, c * CW + 1 : c * CW + 1 + L]
                    if stage == 0:
                        nc.gpsimd.tensor_scalar(tsl, ws[:], 1.0 / KS, None, op0=OP.mult)
                    else:
                        wss = spool.tile([128, L], BF16, name=f"wss{c}", tag="wss", bufs=1)
                        nc.gpsimd.tensor_scalar(wss[:], ws[:], 1.0 / KS, None, op0=OP.mult)
                        nc.gpsimd.tensor_add(tsl, wss[:], tsl)
                    if stage == 2:
                        nc.gpsimd.tensor_copy(tacc[:, c * CW : c * CW + 1],
                                              tacc[:, c * CW + L : c * CW + L + 1])
                        nc.gpsimd.tensor_copy(tacc[:, c * CW + 1025 : c * CW + 1026],
                                              tacc[:, c * CW + 1 : c * CW + 2])

            def ffn_block(b, x3, fcw):
                x3bf = P["l"].tile([128, NCH * L], BF16, name=f"x3bf{b}", tag="x3bf",
                                   bufs=1)
                if b == 0:
                    nc.scalar.activation(x3bf[:], x3[:], AF.Copy)
                else:
                    nc.vector.tensor_copy(x3bf[:], x3[:])
                y3 = ppool.tile([128, NCH * L], F32, name=f"y3{b}", tag="y", bufs=2)
                for half in range(2):
                    h_t = P["l"].tile([128, NXP * 512], BF16, name=f"h{b}{half}",
                                      tag="lbuf16", bufs=2)
                    for xc in range(NXP):
                        pt = ps_tile(f"ph{xc}")
                        for c in range(NCH):
                            nc.tensor.matmul(
                                pt[:], fcw[:, xc * 512 + 128 * c : xc * 512 + 128 * c + 128],
                                x3bf[:, c * L + 512 * half : c * L + 512 * half + 512],
                                start=(c == 0), stop=(c == NCH - 1))
                        nc.scalar.activation(h_t[:, xc * 512 : (xc + 1) * 512], pt[:],
                                             AF.Gelu, bias=bias_pc[:, FC1B + xc : FC1B + xc + 1])
                    for m in range(NCH):
                        pt = ps_tile(f"pf{m}")
                        for xc in range(NXP):
                            nc.tensor.matmul(
                                pt[:],
                                fcw[:, 8192 + m * 2048 + 128 * xc : 8192 + m * 2048 + 128 * xc + 128],
                                h_t[:, xc * 512 : (xc + 1) * 512],
                                start=(xc == 0), stop=(xc == NXP - 1))
                        sl = slice(m * L + 512 * half, m * L + 512 * half + 512)
                        nc.vector.scalar_tensor_tensor(
                            y3[:, sl], pt[:], bias_pc[:, FC2B + m : FC2B + m + 1],
                            x3[:, sl], op0=OP.add, op1=OP.add)
                return y3

            def ln_stats(b, x4):
                sq = P["l"].tile([128, NCH * L], F32R, name=f"sq{b}", tag="lbuf16", bufs=2)
                for c in range(NCH):
                    nc.scalar.activation(sq[:, c * L : (c + 1) * L],
                                         x4[:, c * L : (c + 1) * L], AF.Square)
                mu_ps = pmpool.tile([1, L], F32, name=f"mups{b}", tag="psbig")
                for h in range(2):
                    for c in range(NCH):
                        nc.tensor.matmul(mu_ps[0:1, 512 * h : 512 * h + 512], ones_div,
                                         x4[:, c * L + 512 * h : c * L + 512 * h + 512],
                                         start=(c == 0), stop=(c == NCH - 1))
                mu_r = spool.tile([1, L], F32, name=f"mur{b}", tag="mrow", bufs=1)
                nc.vector.tensor_copy(mu_r[:], mu_ps[:])
                ms_ps = pmpool.tile([1, L], F32, name=f"msps{b}", tag="psbig")
                for h in range(2):
                    for c in range(NCH):
                        nc.tensor.matmul(ms_ps[0:1, 512 * h : 512 * h + 512], ones_div,
                                         sq[:, c * L + 512 * h : c * L + 512 * h + 512],
                                         start=(c == 0), stop=(c == NCH - 1))
                var_r = spool.tile([1, L], F32, name=f"varr{b}", tag="mrev", bufs=1)
                nc.vector.tensor_tensor(var_r[:], mu_r[:], mu_r[:], OP.mult)
                nc.vector.scalar_tensor_tensor(var_r[:], ms_ps[:], 1e-5, var_r[:],
                                               op0=OP.add, op1=OP.subtract)
                nc.scalar.activation(var_r[:], var_r[:], AF.Sqrt)
                rows = spool.tile([1, L], F32R, name=f"rows{b}", tag="c2r", bufs=1)
                rows2 = spool.tile([1, L], F32R, name=f"rows2{b}", tag="rows2", bufs=1)
                with nc.allow_low_precision(reason="istd broadcast is f32r by design"):
                    nc.vector.reciprocal(rows[:], var_r[:])
                nc.vector.tensor_tensor(rows2[:], mu_r[:], rows[:], OP.mult)
                return rows, rows2

            def ln_apply(b, x4, rows, rows2):
                bc = P["l"].tile([128, 2 * L], F32, name=f"bc{b}", tag="lbuf16", bufs=2)
                for h in range(4):
                    bp = ps_tile(f"bc{h}")
                    src_row = rows if h < 2 else rows2
                    nc.tensor.matmul(bp[:], ones_row[:],
                                     src_row[0:1, 512 * (h % 2) : 512 * (h % 2) + 512],
                                     start=True, stop=True)
                    if h % 2 == 0:
                        nc.vector.tensor_copy(bc[:, 512 * h : 512 * h + 512], bp[:])
                    else:
                        nc.scalar.activation(bc[:, 512 * h : 512 * h + 512], bp[:], AF.Copy)
                seas = ppool.tile([128, NCH * L], F32, name=f"seas{b}", tag="y", bufs=2)
                accs = spool.tile([128, NCH], F32, name=f"accs{b}", tag="accs", bufs=2)
                for c in range(NCH):
                    t1 = spool.tile([128, L], F32, name=f"t1{c}", tag="ws", bufs=4)
                    nc.vector.tensor_tensor(t1[:], x4[:, c * L : (c + 1) * L],
                                            bc[:, 0:L], OP.mult)
                    nc.vector.tensor_tensor(t1[:], t1[:], bc[:, L:], OP.subtract)
                    nc.scalar.activation(seas[:, c * L : (c + 1) * L], t1[:], AF.Identity,
                                         bias=bias_pc[:, LNB + c : LNB + c + 1],
                                         scale=bias_pc[:, LNG + c : LNG + c + 1],
                                         accum_out=accs[:, c : c + 1])
                nc.vector.tensor_scalar(accs[:], accs[:], -1.0 / L, None, op0=OP.mult)
                for c in range(NCH):
                    if c % 2 == 0:
                        nc.scalar.activation(seas[:, c * L : (c + 1) * L],
                                             seas[:, c * L : (c + 1) * L], AF.Identity,
                                             bias=accs[:, c : c + 1])
                    else:
                        nc.vector.tensor_scalar(seas[:, c * L : (c + 1) * L],
                                                seas[:, c * L : (c + 1) * L],
                                                accs[:, c : c + 1], None, op0=OP.add)
                return seas

            def seas_out_block(b, seas):
                for a in range(NT):
                    tp = ps_tile(f"tps{a}")
                    for c in range(NCH):
                        nc.tensor.transpose(tp[:, 128 * c : 128 * c + 128],
                                            seas[:, c * L + 128 * a : c * L + 128 * a + 128],
                                            ident[:, :])
                    osb = spool.tile([128, 512], F32, name=f"osb{a}", tag="osb", bufs=2)
                    nc.scalar.activation(osb[:], tp[:], AF.Copy)
                    nc.sync.dma_start(out=seas_out.ap()[b, 128 * a : 128 * a + 128, :],
                                      in_=osb[:])

            def trend_conv(b, wct, rng=None):
                tacc = tacc_t[b]
                for a in (rng if rng is not None else range(NT)):
                    pt = ps_tile(f"ptc{a}")
                    n = 0
                    for j in range(3):
                        for c in range(NCH):
                            nc.tensor.matmul(
                                pt[:],
                                tacc[:, c * CW + 128 * a + j : c * CW + 128 * a + j + 128],
                                wct[:, (j * NCH + c) * F : (j * NCH + c) * F + F],
                                start=(n == 0), stop=(n == 11))
                            n += 1
                    osb = spool.tile([128, 512], F32, name=f"osc{a}", tag="osb", bufs=2)
                    nc.scalar.activation(osb[:], pt[:], AF.Copy)
                    nc.sync.dma_start(out=trend_out.ap()[b, 128 * a : 128 * a + 128, :],
                                      in_=osb[:])

            # ============== attention phase: hand-scheduled emission ==============
            q_t, k_t, v_t, cc_t, sch_t = {}, {}, {}, {}, {}

            def attn_head(b, pfx, wpre=None):
                """proj q,k + band scores + shear round trip."""
                _mark(nc, f"{pfx}{b}.proj_qk")
                xq = x_t[b] if pfx == "ca" else xin_t[b]
                q = P["a"].tile([128, NCH * L], F32R, name=f"q{pfx}{b}", tag="q", bufs=1)
                k = P["a"].tile([128, NCH * L], F32R, name=f"k{pfx}{b}", tag="k", bufs=1)
                proj_el(q, xq, w_in[f"{pfx}_wq"], BQ[pfx], wpre=wpre)
                proj_el(k, xin_t[b], w_in[f"{pfx}_wk"], BK[pfx])
                _mark(nc, f"{pfx}{b}.scores")
                sch_t[b] = scores(b, pfx, q, k)

            def attn_mps(b, pfx):
                _mark(nc, f"{pfx}{b}.mps")
                mp = mps_reduce(b, pfx, sch_t[b])
                _mark(nc, f"{pfx}{b}.softmax")
                cc_t[b] = softmax_c2(b, pfx, mp)

            def attn_projv(b, pfx):
                _mark(nc, f"{pfx}{b}.projv")
                v = P["a"].tile([128, NT * E], BF16, name=f"v{pfx}{b}", tag="v", bufs=1)
                proj_rev(v, (xtr_in if pfx == "sa" else entr_in).ap()[b],
                         w_in[f"{pfx}_wv"])
                v_t[b] = v

            ag_t = {}

            def attn_agg(b, pfx):
                _mark(nc, f"{pfx}{b}.agg")
                ag = P["a"].tile([128, NCH * L], F32R, name=f"agg{pfx}{b}", tag="q", bufs=1)
                agg_block(b, pfx, v_t[b], cc_t[b], ag)
                ag_t[b] = ag

            def attn_projo(b, pfx, stage, resid=None, resid_dram=None):
                _mark(nc, f"{pfx}{b}.projo")
                y = ppool.tile([128, NCH * L], F32, name=f"y{pfx}{b}", tag="y", bufs=2)
                proj_el(y, ag_t[b], w_in[f"{pfx}_wo"], BO[pfx], resid=resid,
                        resid_dram=resid_dram)
                _mark(nc, f"decomp{b}")
                xn = ppool.tile([128, NCH * L], F32R, name=f"x{pfx}{b}", tag="x", bufs=2)
                decomp(b, y, xn, stage)
                x_t[b] = xn

            def attn_tail(b, pfx, stage, resid=None, resid_dram=None):
                attn_agg(b, pfx)
                attn_projo(b, pfx, stage, resid=resid, resid_dram=resid_dram)

            with tc.tile_pool(name="attn", bufs=1) as atpool:
                P["a"] = atpool
                _mark(nc, "init")
                wpre0 = [wm_load(w_in["sa_wq"], m) for m in range(NCH)]
                load_xin(0, xt_in, split=True)
                attn_head(0, "sa", wpre0)
                load_xin(1, xt_in)
                attn_projv(0, "sa")
                attn_mps(0, "sa")
                attn_head(1, "sa")
                attn_tail(0, "sa", 0, resid_dram=xt_in.ap()[0])
                attn_projv(1, "sa")
                load_xin(0, ent_in)
                attn_mps(1, "sa")
                attn_head(0, "ca")                   # needs x_t[0] from decomp0
                attn_tail(1, "sa", 0, resid_dram=xt_in.ap()[1])
                attn_projv(0, "ca")
                load_xin(1, ent_in)
                attn_mps(0, "ca")
                attn_head(1, "ca")
                attn_agg(0, "ca")
                attn_mps(1, "ca")
                attn_projo(0, "ca", 1, resid=x_t[0])
                attn_projv(1, "ca")
                attn_tail(1, "ca", 1, resid=x_t[1])

            with tc.tile_pool(name="late", bufs=1) as ltpool:
                P["l"] = ltpool
                _mark(nc, "ffn_w")
                fcw = ltpool.tile([128, 16384], BF16, name="fcw", tag="bigw", bufs=1)
                for j in range(8):
                    nc.sync.dma_start(out=fcw[:, j * 2048 : (j + 1) * 2048],
                                      in_=fcw_in.ap()[:, j * 2048 : (j + 1) * 2048])
                _mark(nc, "ffn0")
                y0 = ffn_block(0, x_t[0], fcw)
                _mark(nc, "decomp0")
                x40 = ppool.tile([128, NCH * L], F32R, name="x40", tag="x", bufs=2)
                decomp(0, y0, x40, 2)
                x_t[0] = x40
                _mark(nc, "ffn1")
                y1 = ffn_block(1, x_t[1], fcw)
                _mark(nc, "ln0a")
                r0, r20 = ln_stats(0, x40)
                _mark(nc, "decomp1")
                x41 = ppool.tile([128, NCH * L], F32R, name="x41", tag="x", bufs=2)
                decomp(1, y1, x41, 2)
                x_t[1] = x41
                _mark(nc, "trend_w")
                wct = ltpool.tile([128, 12 * F], BF16, name="wctt", tag="bigw", bufs=1)
                nc.sync.dma_start(out=wct[:], in_=wct_in.ap())
                _mark(nc, "trend0")
                trend_conv(0, wct)
                _mark(nc, "ln1a")
                r1, r21 = ln_stats(1, x41)
                _mark(nc, "ln0b")
                seas0 = ln_apply(0, x40, r0, r20)
                _mark(nc, "trend1")
                trend_conv(1, wct, rng=range(0, 4))
                _mark(nc, "ln0out")
                seas_out_block(0, seas0)
                _mark(nc, "ln1b")
                seas1 = ln_apply(1, x41, r1, r21)
                _mark(nc, "trend1b")
                trend_conv(1, wct, rng=range(4, NT))
                _mark(nc, "ln1out")
                seas_out_block(1, seas1)

    nc.compile()
    return nc


def _host_prep(inputs):
    f32 = np.float32
    import ml_dtypes
    x = np.asarray(inputs["x"], f32)
    enc = np.asarray(inputs["enc_output"], f32)
    xt = np.ascontiguousarray(x.transpose(0, 2, 1))
    xtr = np.ascontiguousarray(xt[:, :, ::-1])
    ent = np.ascontiguousarray(enc.transpose(0, 2, 1))
    entr = np.ascontiguousarray(ent[:, :, ::-1])

    shared = {}
    for p in ("sa", "ca"):
        for nme in ("wq", "wk", "wv", "wo"):
            shared[f"{p}_{nme}"] = np.ascontiguousarray(np.asarray(inputs[f"{p}_{nme}"], f32))
    fc1 = np.asarray(inputs["fc1_w"], f32).reshape(NCH, 128, XP)       # (c, p, xp)
    fc2 = np.asarray(inputs["fc2_w"], f32).reshape(NXP, 128, E)        # (xc, p, e)
    fcw = np.zeros((128, 16384), ml_dtypes.bfloat16)
    fcw[:, :8192] = fc1.reshape(NCH, 128, NXP, 128).transpose(1, 2, 0, 3) \
        .reshape(128, 8192).astype(ml_dtypes.bfloat16)
    fcw[:, 8192:] = fc2.reshape(NXP, 128, NCH, 128).transpose(1, 2, 0, 3) \
        .reshape(128, 8192).astype(ml_dtypes.bfloat16)
    shared["fcw"] = fcw
    tw = np.asarray(inputs["trend_w"], f32)
    # [p, (j c), f]: row (j*4+c)*128+p of the (3E, F) matrix = w[(j e)], e=128c+p
    wct = tw.transpose(2, 1, 0).reshape(3 * E, F)                      # [(j e), f]
    shared["wct"] = np.ascontiguousarray(
        wct.reshape(12, 128, F).transpose(1, 0, 2).reshape(128, 12 * F)
    ).astype(ml_dtypes.bfloat16)

    def pc(v, nch=4):
        return np.ascontiguousarray(np.asarray(v, f32).reshape(nch, 128).T)

    # fold bv into bo: out = Wo^T (agg + bv) + bo = Wo^T agg + (bv @ Wo + bo)
    sa_bo2 = np.asarray(inputs["sa_bv"], f32) @ np.asarray(inputs["sa_wo"], f32) \
        + np.asarray(inputs["sa_bo"], f32)
    ca_bo2 = np.asarray(inputs["ca_bv"], f32) @ np.asarray(inputs["ca_wo"], f32) \
        + np.asarray(inputs["ca_bo"], f32)
    shared["bias_pc"] = np.ascontiguousarray(np.concatenate([
        pc(inputs["sa_bq"]), pc(inputs["sa_bk"]), pc(sa_bo2),
        pc(inputs["ca_bq"]), pc(inputs["ca_bk"]), pc(ca_bo2),
        pc(inputs["fc2_b"]), pc(inputs["ln_g"]), pc(inputs["ln_b"]),
        pc(inputs["fc1_b"], 16),
    ], axis=1))
    ramp = np.zeros(16, f32)
    ramp[:13] = np.arange(12, -1, -1)
    shared["consts"] = np.ascontiguousarray(np.concatenate([
        np.eye(128, dtype=f32),                       # ident 0:128
        np.eye(128, dtype=f32)[::-1],                 # jmat 128:256
        np.tile(ramp, (128, 1)),                      # rampl 256:272
        np.ones((128, 12), f32),                      # ones12 272:284
        shared.pop("bias_pc"),                        # bias_pc 284:336
    ], axis=1))
    shared["constr"] = np.ascontiguousarray(np.concatenate([
        np.eye(128, dtype=f32),                       # identr 0:128
        np.full((128, 1), 1.0 / E, f32),              # ones_div 128:129
    ], axis=1))
    shared["ones_row"] = np.ones((1, 128), f32)

    in_maps = []
    for core in range(NCORES):
        s = slice(core * BPC, (core + 1) * BPC)
        m = dict(shared)
        m["xt"] = np.ascontiguousarray(xt[s])
        m["xtr"] = np.ascontiguousarray(xtr[s])
        m["ent"] = np.ascontiguousarray(ent[s])
        m["entr"] = np.ascontiguousarray(entr[s])
        in_maps.append(m)
    return in_maps


_LAST = {}


def kernel(**inputs):
    from concourse.bass_utils import run_bass_kernel_spmd

    nc = _build()
    in_maps = _host_prep(inputs)
    res = run_bass_kernel_spmd(nc, in_maps, core_ids=list(range(NCORES)),
                               **_LAST.get("kwargs", {}))
    _LAST["res"] = res
    seasonal = np.concatenate([res.results[c]["seasonal"] for c in range(NCORES)], axis=0)
    trend = np.concatenate([res.results[c]["trend"] for c in range(NCORES)], axis=0)
    return seasonal, trend
